# revision 28
# baseline (speedup 1.0000x reference)
"""Trainium2 Bass kernel for nn_DiscreteDiT (DiT backbone + pairwise MLP + Sinkhorn).

Self-contained: hardcodes all shapes. Shards batch B=8 across 8 NeuronCores
(one batch element per core, params replicated / host-prefolded per batch).
"""
import sys

for _p in ("/opt/trn_rl_repo", "/root/.axon_site/_ro/trn_rl_repo"):
    if _p not in sys.path:
        sys.path.insert(0, _p)

import numpy as np

import concourse.bacc as bacc
import concourse.mybir as mybir
import concourse.tile as tile
from concourse.bass_utils import run_bass_kernel_spmd
from concourse.masks import make_identity

F32 = mybir.dt.float32
F32R = mybir.dt.float32r
I32 = mybir.dt.int32
AF = mybir.ActivationFunctionType
ALU = mybir.AluOpType
AX = mybir.AxisListType

N = 192          # nodes / tokens
ENC = 1728       # encoding dim
D = 256          # d_model
LAYERS = 4
HEADS = 4
HD = 64
PROJ = 256
MLP_H = 256      # pair MLP hidden
BB_H = 1024      # backbone MLP hidden
SINK_ITERS = 20
B = 8

TCH = [(0, 128), (128, 64)]            # token chunks (start, size)
ECH = [(i * 128, 128) for i in range(13)] + [(1664, 64)]   # ENC chunks
NEC = len(ECH)
IBLK = 16                               # pair-stage i block

_PROFILE = {}


# ----------------------------------------------------------------- host prep
def _silu(x):
    return x / (1.0 + np.exp(-x))


def _np(a):
    return np.asarray(a, dtype=np.float64)


def _host_prep(x, timesteps, params):
    """Returns list of 8 per-core input dicts."""
    x = np.asarray(x, np.float64)
    ts = np.asarray(timesteps).astype(np.float64)
    p = params

    half = D // 2
    # match reference: freqs computed in fp32, args/sin/cos then in fp64 for accuracy
    freqs = np.exp(-np.log(np.float32(10000.0)) * np.arange(half, dtype=np.float32)
                   / np.float32(half)).astype(np.float32).astype(np.float64)
    args = ts[:, None] * freqs[None, :]
    temb = np.concatenate([np.sin(args), np.cos(args)], axis=-1)
    t = _silu(temb @ _np(p["t_mlp1"]["w"]) + _np(p["t_mlp1"]["b"]))
    t = t @ _np(p["t_mlp2"]["w"]) + _np(p["t_mlp2"]["b"])         # [B, D]
    st = _silu(t)

    w_in = _np(p["in_proj"]["w"])                                  # [ENC, D]
    pos_full = _np(p["pos_emb"]) + _np(p["in_proj"]["b"])[None, :]  # [N, D]

    fsmod = st @ _np(p["final_mod"]["w"]) + _np(p["final_mod"]["b"])  # [B, 2D]
    fs, fsc = fsmod[:, :D], fsmod[:, D:]

    w_f = _np(p["final"]["w"])
    b_f = _np(p["final"]["b"])

    w1 = _np(p["pair"][0]["w"])     # [2*PROJ, MLP_H]
    b1 = _np(p["pair"][0]["b"])
    w1a, w1b = w1[:PROJ], w1[PROJ:]
    w2p = _np(p["pair"][1]["w"])    # [MLP_H, MLP_H]
    b2p = _np(p["pair"][1]["b"])
    w3 = _np(p["pair_out"]["w"]).reshape(-1)   # [MLP_H]
    b3 = float(np.asarray(p["pair_out"]["b"]).reshape(-1)[0])

    w3blk = np.zeros((128, 8, 4), np.float64)
    for ii in range(4):
        for mc in range(2):
            w3blk[:, ii * 2 + mc, ii] = w3[mc * 128:(mc + 1) * 128]

    eye = np.eye(N, dtype=np.float64)
    mask_half = 0.5 * (1.0 - eye)
    diagm = b3 * (1.0 - eye) - 1e9 * eye

    lw = []
    for l in range(LAYERS):
        lp = p["layers"][l]
        mod = st @ _np(lp["mod"]["w"]) + _np(lp["mod"]["b"])       # [B, 6D]
        s1, sc1, g1, s2, sc2, g2 = np.split(mod, 6, axis=-1)
        lw.append(dict(
            wq=_np(lp["wq"]["w"]), bq=_np(lp["wq"]["b"]),
            wk=_np(lp["wk"]["w"]), bk=_np(lp["wk"]["b"]),
            wv=_np(lp["wv"]["w"]), bv=_np(lp["wv"]["b"]),
            wo=_np(lp["wo"]["w"]), bo=_np(lp["wo"]["b"]),
            w1=_np(lp["mlp1"]["w"]), b1=_np(lp["mlp1"]["b"]),
            w2=_np(lp["mlp2"]["w"]), b2=_np(lp["mlp2"]["b"]),
            s1=s1, sc1=sc1, g1=g1, s2=s2, sc2=sc2, g2=g2,
        ))

    in_maps = []
    sc = np.float32(1.0 / np.sqrt(HD))
    for b in range(B):
        m = {
            "xt": x[b].T,                                    # [ENC, N]
            "w_in": w_in,
            "pos": pos_full,
            "wf": (1.0 + fsc[b])[:, None] * w_f,
            "cf": fs[b] @ w_f + b_f,
            "w1a": w1a, "w1b": w1b, "cu": b1,
            "w2p": w2p, "b2p": b2p, "w3blk": w3blk,
            "mask_half": mask_half, "diagm": diagm,
            "ones_r": np.ones(2 * N, np.float32),
            "b2p_r": np.asarray(b2p, np.float32),
        }
        for l, W in enumerate(lw):
            d1 = (1.0 + W["sc1"][b])[:, None]
            d2 = (1.0 + W["sc2"][b])[:, None]
            m[f"wq{l}"] = d1 * W["wq"] * sc
            m[f"cq{l}"] = (W["s1"][b] @ W["wq"] + W["bq"]) * sc
            m[f"wk{l}"] = d1 * W["wk"]
            m[f"ck{l}"] = W["s1"][b] @ W["wk"] + W["bk"]
            m[f"wv{l}"] = d1 * W["wv"]
            m[f"cv{l}"] = W["s1"][b] @ W["wv"] + W["bv"]
            m[f"wo{l}"] = W["wo"] * W["g1"][b][None, :]
            m[f"co{l}"] = W["bo"] * W["g1"][b]
            m[f"w1{l}"] = d2 * W["w1"]
            m[f"c1{l}"] = W["s2"][b] @ W["w1"] + W["b1"]
            m[f"w2{l}"] = W["w2"] * W["g2"][b][None, :]
            m[f"c2{l}"] = W["b2"] * W["g2"][b]
        m = {k: np.ascontiguousarray(v, dtype=np.float32) for k, v in m.items()}
        in_maps.append(m)
    return in_maps


# ------------------------------------------------------------- device program
def build_program(_debug=False):
    nc = bacc.Bacc("TRN2", target_bir_lowering=False, debug=False)

    def din(name, shape):
        return nc.dram_tensor(name, list(shape), F32, kind="ExternalInput").ap()

    xt_d = nc.dram_tensor("xt", [ENC, N], F32R, kind="ExternalInput").ap()
    win_d = nc.dram_tensor("w_in", [ENC, D], F32R, kind="ExternalInput").ap()
    pos_d = din("pos", (N, D))
    wf_d = nc.dram_tensor("wf", [D, D], F32R, kind="ExternalInput").ap()
    cf_d = din("cf", (D,))
    w1a_d = nc.dram_tensor("w1a", [PROJ, MLP_H], F32R, kind="ExternalInput").ap()
    w1b_d = nc.dram_tensor("w1b", [PROJ, MLP_H], F32R, kind="ExternalInput").ap()
    cu_d = din("cu", (MLP_H,))
    w2p_d = nc.dram_tensor("w2p", [MLP_H, MLP_H], F32R, kind="ExternalInput").ap()
    b2p_d = din("b2p", (MLP_H,))
    w3blk_d = nc.dram_tensor("w3blk", [128, 8, 4], F32R, kind="ExternalInput").ap()
    onesr_d = nc.dram_tensor("ones_r", [2 * N], F32R, kind="ExternalInput").ap()
    b2pr_d = nc.dram_tensor("b2p_r", [MLP_H], F32R, kind="ExternalInput").ap()
    maskh_d = din("mask_half", (N, N))
    diagm_d = din("diagm", (N, N))
    LW = []
    for l in range(LAYERS):
        ent = {}
        for k, shp in [("wq", (D, D)), ("cq", (D,)), ("wk", (D, D)), ("ck", (D,)),
                       ("wv", (D, D)), ("cv", (D,)), ("wo", (D, D)), ("co", (D,)),
                       ("w1", (D, BB_H)), ("c1", (BB_H,)), ("w2", (BB_H, D)), ("c2", (D,))]:
            dt_ = F32R if k in ("wq", "wk", "w1", "wv", "wo", "w2") else F32
            ent[k] = nc.dram_tensor(f"{k}{l}", list(shp), dt_, kind="ExternalInput").ap()
        LW.append(ent)

    logits_d = nc.dram_tensor("logits", [N, N], F32, kind="ExternalOutput").ap()
    x0_d = nc.dram_tensor("x0", [N, N], F32, kind="ExternalOutput").ap()
    DBG = {}
    if _debug:
        for nm, shp in [("dbg_h0", (N, D)), ("dbg_h1", (N, D)), ("dbg_h2", (N, D)),
                        ("dbg_h3", (N, D)), ("dbg_h4", (N, D)),
                        ("dbg_hfT", (D, N)), ("dbg_uT", (D, N)), ("dbg_vT", (D, N)),
                        ("dbg_L", (N, N))]:
            DBG[nm] = nc.dram_tensor(nm, list(shp), F32, kind="ExternalOutput").ap()

    def dump_h(nm):
        if _debug:
            for t, (toff, tsz) in enumerate(TCH):
                nc.sync.dma_start(out=DBG[nm][toff:toff + tsz, :], in_=h_sb[0:tsz, t, :])

    def dump_T(nm, tl):
        if _debug:
            for c in range(2):
                nc.sync.dma_start(out=DBG[nm][c * 128:(c + 1) * 128, :], in_=tl[:, c, :])

    col2 = lambda a: a.rearrange("(c p) -> p c", p=128)     # [2X*128] -> [128, 2X]
    row = lambda a: a.rearrange("(o p) -> o p", o=1)        # [X] -> [1, X]

    with tile.TileContext(nc) as tc:
        with nc.allow_low_precision(reason="f32r intermediates; validated end-to-end"), \
             tc.tile_pool(name="cons", bufs=1) as cons, \
             tc.tile_pool(name="sb", bufs=1) as sb:

            ident = cons.tile([128, 128], F32)
            make_identity(nc, ident)
            ones_row = cons.tile([1, 192], F32)
            nc.vector.memset(ones_row, 1.0)
            ones_row_r = cons.tile([1, 2 * N], F32R)
            nc.sync.dma_start(out=ones_row_r, in_=onesr_d.rearrange("(o p) -> o p", o=1))
            ones_col = cons.tile([128, 1], F32)
            nc.vector.memset(ones_col, 1.0)
            epst = cons.tile([128, 1], F32)
            nc.vector.memset(epst, 1e-6)

            h_sb = cons.tile([128, 2, D], F32)

            def layer_norm(wp, psp, use_newton=False):
                """LN of h_sb -> transposed hlnT [128, 2kc, N]."""
                mv = []
                for t, (toff, tsz) in enumerate(TCH):
                    st_t = wp.tile([128, 6], F32, tag="bnst")
                    nc.vector.bn_stats(out=st_t[0:tsz, :], in_=h_sb[0:tsz, t, :])
                    mv_t = wp.tile([128, 2], F32, tag=f"bnmv{t}")
                    nc.vector.bn_aggr(out=mv_t[0:tsz, :], in_=st_t[0:tsz, :])
                    mv.append(mv_t)
                vpack = wp.tile([128, 2], F32, tag="vpack")
                nc.vector.tensor_copy(out=vpack[:, 0:1], in_=mv[0][:, 1:2])
                nc.vector.tensor_copy(out=vpack[0:64, 1:2], in_=mv[1][0:64, 1:2])
                rstd = wp.tile([128, 2], F32, tag="rstd")
                if not use_newton:
                    lnv = wp.tile([128, 2], F32, tag="lnv")
                    nc.scalar.activation(out=lnv, in_=vpack, func=AF.Ln, bias=epst, scale=1.0)
                    nc.scalar.activation(out=rstd, in_=lnv, func=AF.Exp, bias=0.0, scale=-0.5)
                else:
                    a = wp.tile([128, 2], F32, tag="nta")
                    nc.vector.tensor_scalar(out=a, in0=vpack, scalar1=1e-6, scalar2=None,
                                            op0=ALU.add)
                    xh = wp.tile([128, 2], F32, tag="ntxh")
                    nc.vector.tensor_scalar(out=xh, in0=a, scalar1=0.5, scalar2=None,
                                            op0=ALU.mult)
                    ti = wp.tile([128, 2], I32, tag="nti")
                    nc.vector.tensor_scalar(out=ti, in0=a.bitcast(I32), scalar1=1,
                                            scalar2=None, op0=ALU.logical_shift_right)
                    nc.vector.tensor_scalar(out=ti, in0=ti, scalar1=-1, scalar2=None,
                                            op0=ALU.bitwise_xor)
                    nc.vector.tensor_scalar(out=ti, in0=ti, scalar1=0x5f3759df + 1,
                                            scalar2=None, op0=ALU.add)
                    y = rstd
                    nc.vector.tensor_copy(out=y, in_=ti.bitcast(F32))
                    t1 = wp.tile([128, 2], F32, tag="ntt1")
                    for _ in range(3):
                        nc.vector.tensor_tensor(out=t1, in0=y, in1=y, op=ALU.mult)
                        nc.vector.tensor_tensor(out=t1, in0=t1, in1=xh, op=ALU.mult)
                        nc.vector.tensor_scalar(out=t1, in0=t1, scalar1=-1.0, scalar2=1.5,
                                                op0=ALU.mult, op1=ALU.add)
                        nc.vector.tensor_tensor(out=y, in0=y, in1=t1, op=ALU.mult)
                hln = wp.tile([128, 2, D], F32, tag="hln")
                for t, (toff, tsz) in enumerate(TCH):
                    nc.vector.tensor_scalar(out=hln[0:tsz, t, :], in0=h_sb[0:tsz, t, :],
                                            scalar1=mv[t][0:tsz, 0:1],
                                            scalar2=rstd[0:tsz, t:t + 1],
                                            op0=ALU.subtract, op1=ALU.mult)
                pt = psp.tile([128, 2, 256], F32, tag="ps")
                for kc in range(2):
                    for t, (toff, tsz) in enumerate(TCH):
                        nc.tensor.transpose(pt[0:128, kc, toff:toff + tsz],
                                            hln[0:tsz, t, kc * 128:(kc + 1) * 128],
                                            ident[0:tsz, 0:tsz])
                hlnT = wp.tile([128, 2, 256], F32R, tag="hlnT")
                nc.vector.tensor_copy(out=hlnT[:, :, 0:N], in_=pt[:, :, 0:N])
                return hlnT

            # ================= in_proj =================
            with tc.tile_pool(name="inp", bufs=1) as inp, \
                 tc.tile_pool(name="psA", bufs=2, space="PSUM") as psA:
                xt_sb = inp.tile([128, NEC, N], F32R)
                win_sb = inp.tile([128, NEC, D], F32R)
                pos_sb = inp.tile([128, 2, D], F32)
                for c, (off, sz) in enumerate(ECH):
                    nc.sync.dma_start(out=xt_sb[0:sz, c, :], in_=xt_d[off:off + sz, :])
                    nc.sync.dma_start(out=win_sb[0:sz, c, :], in_=win_d[off:off + sz, :])
                for t, (toff, tsz) in enumerate(TCH):
                    nc.sync.dma_start(out=pos_sb[0:tsz, t, :], in_=pos_d[toff:toff + tsz, :])
                for t, (toff, tsz) in enumerate(TCH):
                    ph = psA.tile([128, D], F32, tag="ph")
                    for c, (off, sz) in enumerate(ECH):
                        if tsz == 128:
                            nc.tensor.matmul(ph[0:tsz, :], xt_sb[0:sz, c, toff:toff + tsz],
                                             win_sb[0:sz, c, :],
                                             start=(c == 0), stop=(c == NEC - 1))
                        else:
                            nc.tensor.matmul(ph[0:tsz, :],
                                             xt_sb.bitcast(F32)[0:sz, c, toff:toff + tsz],
                                             win_sb.bitcast(F32)[0:sz, c, :],
                                             start=(c == 0), stop=(c == NEC - 1))
                    nc.vector.tensor_tensor(out=h_sb[0:tsz, t, :], in0=ph[0:tsz, :],
                                            in1=pos_sb[0:tsz, t, :], op=ALU.add)
                dump_h("dbg_h0")

            # ================= backbone =================
            with tc.tile_pool(name="wpool", bufs=2) as wpool, \
                 tc.tile_pool(name="tpool", bufs=2) as tpool, \
                 tc.tile_pool(name="psB", bufs=6, space="PSUM") as psB, \
                 tc.tile_pool(name="psS", bufs=2, space="PSUM") as psSm:
                for l in range(LAYERS):
                    wd = LW[l]
                    wq = wpool.tile([128, 2, D], F32R, tag="wq")
                    wk = wpool.tile([128, 2, D], F32R, tag="wk")
                    wv = wpool.tile([128, 2, D], F32R, tag="wv")
                    wo = wpool.tile([128, 2, D], F32R, tag="wo")
                    w1 = wpool.tile([128, 2, BB_H], F32R, tag="w1")
                    w2 = wpool.tile([128, 8, D], F32R, tag="w2")
                    for w_sb, w_dr in [(wq, wd["wq"]), (wk, wd["wk"]), (wv, wd["wv"]),
                                       (wo, wd["wo"]), (w1, wd["w1"]), (w2, wd["w2"])]:
                        nc.sync.dma_start(out=w_sb, in_=w_dr.rearrange("(c p) m -> p c m", p=128))
                    cq = wpool.tile([128, 2], F32, tag="cq")
                    nc.sync.dma_start(out=cq, in_=col2(wd["cq"]))
                    ck = wpool.tile([128, 2], F32, tag="ck")
                    nc.sync.dma_start(out=ck, in_=col2(wd["ck"]))
                    c1 = wpool.tile([128, 8], F32, tag="c1")
                    nc.sync.dma_start(out=c1, in_=col2(wd["c1"]))
                    cv = wpool.tile([1, D], F32, tag="cv")
                    nc.sync.dma_start(out=cv, in_=row(wd["cv"]))
                    co = wpool.tile([1, D], F32, tag="co")
                    nc.sync.dma_start(out=co, in_=row(wd["co"]))
                    c2 = wpool.tile([1, D], F32, tag="c2")
                    nc.sync.dma_start(out=c2, in_=row(wd["c2"]))

                    # --- attention ---
                    hlnT = layer_norm(tpool, psB)
                    qT = tpool.tile([128, 2, N], F32, tag="qT")
                    kT = tpool.tile([128, 2, N], F32, tag="kT")
                    for w_sb, c_sb, out_sb in [(wq, cq, qT), (wk, ck, kT)]:
                        pq = psB.tile([128, 2, 256], F32, tag="ps")
                        for mc in range(2):
                            for kc in range(2):
                                nc.tensor.matmul(pq[:, mc, :],
                                                 w_sb[:, kc, mc * 128:(mc + 1) * 128],
                                                 hlnT[:, kc, :], start=(kc == 0), stop=(kc == 1))
                        for mc in range(2):
                            nc.vector.tensor_scalar(out=out_sb[:, mc, :], in0=pq[:, mc, 0:N],
                                                    scalar1=c_sb[:, mc:mc + 1], scalar2=None,
                                                    op0=ALU.add)
                    pv = psB.tile([128, 2, 256], F32, tag="ps")
                    for t, (toff, tsz) in enumerate(TCH):
                        for kc in range(2):
                            if tsz == 128:
                                nc.tensor.matmul(pv[0:tsz, t, :],
                                                 hlnT[:, kc, toff:toff + tsz],
                                                 wv[:, kc, :], start=(kc == 0), stop=False)
                            else:
                                nc.tensor.matmul(pv[0:tsz, t, :],
                                                 hlnT.bitcast(F32)[:, kc, toff:toff + tsz],
                                                 wv.bitcast(F32)[:, kc, :],
                                                 start=(kc == 0), stop=False)
                        nc.tensor.matmul(pv[0:tsz, t, :], ones_row[0:1, 0:tsz], cv,
                                         start=False, stop=True)
                    v_sb = tpool.tile([128, 2, D], F32, tag="v_sb")
                    nc.vector.tensor_copy(out=v_sb, in_=pv)

                    ET = tpool.tile([128, 2, HEADS, N], F32, tag="ET")
                    for hh in range(HEADS):
                        hp, mc = 64 * (hh % 2), hh // 2
                        pss = psB.tile([128, 2, 256], F32, tag="ps")
                        for t2, (t2off, t2sz) in enumerate(TCH):
                            nc.tensor.matmul(pss[0:t2sz, t2, 0:N],
                                             kT[hp:hp + 64, mc, t2off:t2off + t2sz],
                                             qT[hp:hp + 64, mc, :], start=True, stop=True)
                        nc.scalar.activation(out=ET[:, :, hh, :], in_=pss[:, :, 0:N],
                                             func=AF.Exp, bias=0.0, scale=1.0)
                    for hpair in range(2):
                        prs = psSm.tile([1, 2, 256], F32, tag="pr")
                        for hi in range(2):
                            hh = hpair * 2 + hi
                            for t2, (t2off, t2sz) in enumerate(TCH):
                                nc.tensor.matmul(prs[0:1, hi, 0:N], ones_col[0:t2sz, 0:1],
                                                 ET[0:t2sz, t2, hh, :],
                                                 start=(t2 == 0), stop=(t2 == 1))
                        rsr = tpool.tile([1, 2, N], F32, tag="rsr")
                        nc.vector.reciprocal(out=rsr, in_=prs[0:1, :, 0:N])
                        pbc = psB.tile([128, 2, 256], F32, tag="ps")
                        for hi in range(2):
                            nc.tensor.matmul(pbc[:, hi, 0:N], ones_row[0:1, 0:128],
                                             rsr[0:1, hi, :], start=True, stop=True)
                        for t2 in range(2):
                            nc.vector.tensor_tensor(
                                out=ET[:, t2, hpair * 2:hpair * 2 + 2, :],
                                in0=ET[:, t2, hpair * 2:hpair * 2 + 2, :],
                                in1=pbc[:, :, 0:N], op=ALU.mult)
                    po = psB.tile([128, 2, 256], F32, tag="ps")
                    for hh in range(HEADS):
                        hp, s = 64 * (hh % 2), hh // 2
                        for t2, (t2off, t2sz) in enumerate(TCH):
                            nc.tensor.matmul(po[hp:hp + 64, s, 0:N],
                                             v_sb[0:t2sz, t2, hh * 64:hh * 64 + 64],
                                             ET[0:t2sz, t2, hh, :],
                                             start=(t2 == 0), stop=(t2 == 1),
                                             tile_position=(0, hp))
                    oT = tpool.tile([128, 2, N], F32R, tag="oT")
                    nc.vector.tensor_copy(out=oT, in_=po[:, :, 0:N])
                    pa = psB.tile([128, 2, 256], F32, tag="ps")
                    for t, (toff, tsz) in enumerate(TCH):
                        for kc in range(2):
                            if tsz == 128:
                                nc.tensor.matmul(pa[0:tsz, t, :], oT[:, kc, toff:toff + tsz],
                                                 wo[:, kc, :], start=(kc == 0), stop=False)
                            else:
                                nc.tensor.matmul(pa[0:tsz, t, :],
                                                 oT.bitcast(F32)[:, kc, toff:toff + tsz],
                                                 wo.bitcast(F32)[:, kc, :],
                                                 start=(kc == 0), stop=False)
                        nc.tensor.matmul(pa[0:tsz, t, :], ones_row[0:1, 0:tsz], co,
                                         start=False, stop=True)
                    for t, (toff, tsz) in enumerate(TCH):
                        nc.vector.tensor_tensor(out=h_sb[0:tsz, t, :], in0=h_sb[0:tsz, t, :],
                                                in1=pa[0:tsz, t, :], op=ALU.add)

                    # --- mlp ---
                    hln2T = layer_norm(tpool, psB)
                    gm1T = tpool.tile([128, 8, N], F32R, tag="gm1T")
                    for mc in range(8):
                        pm = psB.tile([128, 256], F32, tag="ps")
                        for kc in range(2):
                            nc.tensor.matmul(pm[:, :],
                                             w1[:, kc, mc * 128:(mc + 1) * 128],
                                             hln2T[:, kc, :],
                                             start=(kc == 0), stop=(kc == 1))
                        nc.scalar.activation(out=gm1T[:, mc, :], in_=pm[:, 0:N],
                                             func=AF.Gelu_apprx_tanh,
                                             bias=c1[:, mc:mc + 1], scale=1.0)
                    pa2 = psB.tile([128, 2, 256], F32, tag="ps")
                    for t, (toff, tsz) in enumerate(TCH):
                        for kc in range(8):
                            if tsz == 128:
                                nc.tensor.matmul(pa2[0:tsz, t, :], gm1T[:, kc, toff:toff + tsz],
                                                 w2[:, kc, :], start=(kc == 0), stop=False)
                            else:
                                nc.tensor.matmul(pa2[0:tsz, t, :],
                                                 gm1T.bitcast(F32)[:, kc, toff:toff + tsz],
                                                 w2.bitcast(F32)[:, kc, :],
                                                 start=(kc == 0), stop=False)
                        nc.tensor.matmul(pa2[0:tsz, t, :], ones_row[0:1, 0:tsz], c2,
                                         start=False, stop=True)
                    for t, (toff, tsz) in enumerate(TCH):
                        nc.vector.tensor_tensor(out=h_sb[0:tsz, t, :], in0=h_sb[0:tsz, t, :],
                                                in1=pa2[0:tsz, t, :], op=ALU.add)
                    dump_h(f"dbg_h{l + 1}")

                # ============ final LN + projections (inside backbone pools) ============
                wf_sb = sb.tile([128, 2, D], F32R)
                nc.sync.dma_start(out=wf_sb, in_=wf_d.rearrange("(c p) m -> p c m", p=128))
                cf_sb = sb.tile([128, 2], F32)
                nc.sync.dma_start(out=cf_sb, in_=col2(cf_d))
                w1a_sb = sb.tile([128, 2, MLP_H], F32R)
                nc.sync.dma_start(out=w1a_sb, in_=w1a_d.rearrange("(c p) m -> p c m", p=128))
                w1b_sb = sb.tile([128, 2, MLP_H], F32R)
                nc.sync.dma_start(out=w1b_sb, in_=w1b_d.rearrange("(c p) m -> p c m", p=128))
                cu_sb = sb.tile([128, 2], F32)
                nc.sync.dma_start(out=cu_sb, in_=col2(cu_d))

                hlnfT = layer_norm(tpool, psB, use_newton=True)
                uT = sb.tile([128, 2, N], F32)
                vT = sb.tile([128, 2, N], F32)
                hfT = sb.tile([128, 2, 256], F32R)
                pf = psB.tile([128, 2, 256], F32, tag="ps")
                for mc in range(2):
                    for kc in range(2):
                        nc.tensor.matmul(pf[:, mc, :], wf_sb[:, kc, mc * 128:(mc + 1) * 128],
                                         hlnfT[:, kc, :], start=(kc == 0), stop=(kc == 1))
                for mc in range(2):
                    nc.vector.tensor_scalar(out=hfT[:, mc, 0:N], in0=pf[:, mc, 0:N],
                                            scalar1=cf_sb[:, mc:mc + 1], scalar2=None,
                                            op0=ALU.add)
                pu = psB.tile([128, 2, 256], F32, tag="ps")
                pvv = psB.tile([128, 2, 256], F32, tag="ps")
                for mc in range(2):
                    for kc in range(2):
                        nc.tensor.matmul(pu[:, mc, :], w1a_sb[:, kc, mc * 128:(mc + 1) * 128],
                                         hfT[:, kc, :], start=(kc == 0), stop=(kc == 1))
                        nc.tensor.matmul(pvv[:, mc, :], w1b_sb[:, kc, mc * 128:(mc + 1) * 128],
                                         hfT[:, kc, :], start=(kc == 0), stop=(kc == 1))
                for mc in range(2):
                    nc.vector.tensor_scalar(out=uT[:, mc, :], in0=pu[:, mc, 0:N],
                                            scalar1=cu_sb[:, mc:mc + 1], scalar2=None,
                                            op0=ALU.add)
                nc.vector.tensor_copy(out=vT, in_=pvv[:, :, 0:N])
                dump_T("dbg_hfT", hfT)
                dump_T("dbg_uT", uT)
                dump_T("dbg_vT", vT)

            # ================= pairwise MLP =================
            w2p_sb = sb.tile([128, 2, MLP_H], F32R)
            nc.sync.dma_start(out=w2p_sb, in_=w2p_d.rearrange("(c p) m -> p c m", p=128))
            b2p_sb = sb.tile([128, 2], F32)
            nc.sync.dma_start(out=b2p_sb, in_=col2(b2p_d))
            b2pr_sb = sb.tile([1, MLP_H], F32R)
            nc.sync.dma_start(out=b2pr_sb, in_=b2pr_d.rearrange("(o p) -> o p", o=1))
            w3b_sb = sb.tile([128, 8, 4], F32R)
            nc.sync.dma_start(out=w3b_sb, in_=w3blk_d)
            maskh_sb = sb.tile([128, 2, N], F32)
            diagm_sb = sb.tile([128, 2, N], F32)
            for t, (toff, tsz) in enumerate(TCH):
                nc.sync.dma_start(out=maskh_sb[0:tsz, t, :], in_=maskh_d[toff:toff + tsz, :])
                nc.sync.dma_start(out=diagm_sb[0:tsz, t, :], in_=diagm_d[toff:toff + tsz, :])

            L_sb = sb.tile([128, 2, N], F32)
            with tc.tile_pool(name="zp", bufs=2) as zp, \
                 tc.tile_pool(name="gzp", bufs=2) as gzp, \
                 tc.tile_pool(name="g2p", bufs=3) as g2p, \
                 tc.tile_pool(name="sc4", bufs=3) as sc4p, \
                 tc.tile_pool(name="pps", bufs=3, space="PSUM") as pps, \
                 tc.tile_pool(name="plt", bufs=2, space="PSUM") as plt:
                for ib in range(N // IBLK):
                    z = zp.tile([128, 2, IBLK * N], F32)
                    gz = gzp.tile([128, 2, IBLK * N], F32R)
                    for ii in range(IBLK):
                        i = ib * IBLK + ii
                        for kc in range(2):
                            nc.vector.tensor_scalar(out=z[:, kc, ii * N:(ii + 1) * N],
                                                    in0=vT[:, kc, :],
                                                    scalar1=uT[:, kc, i:i + 1],
                                                    scalar2=None, op0=ALU.add)
                    nc.scalar.activation(out=gz, in_=z,
                                         func=AF.Gelu_apprx_tanh, bias=0.0, scale=1.0)
                    for g4 in range(IBLK // 4):
                        g2sb = g2p.tile([128, 2, 4, 256], F32R)
                        for pp in range(2):
                            # [128, mc, 512-pad]: one accumulation group per bank,
                            # N=384 covers two consecutive i's in one matmul
                            ps2 = pps.tile([128, 2, 512], F32)
                            iioff = (g4 * 4 + pp * 2) * N
                            for mc in range(2):
                                for kc in range(2):
                                    nc.tensor.matmul(
                                        ps2[:, mc, 0:2 * N],
                                        w2p_sb[:, kc, mc * 128:(mc + 1) * 128],
                                        gz[:, kc, iioff:iioff + 2 * N],
                                        start=(kc == 0), stop=False)
                                nc.tensor.matmul(
                                    ps2[:, mc, 0:2 * N],
                                    b2pr_sb[0:1, mc * 128:(mc + 1) * 128],
                                    ones_row_r[0:1, 0:2 * N],
                                    start=False, stop=True)
                            nc.scalar.activation(
                                out=g2sb[:, :, pp * 2:pp * 2 + 2, 0:N],
                                in_=ps2[:, :, 0:2 * N].rearrange(
                                    "p a (b n) -> p a b n", n=N),
                                func=AF.Gelu_apprx_tanh, bias=0.0, scale=1.0)
                        lt = plt.tile([4, 256], F32)
                        for kc8 in range(8):
                            ii, mc = kc8 // 2, kc8 % 2
                            nc.tensor.matmul(lt[0:4, :], w3b_sb[:, kc8, :],
                                             g2sb[:, mc, ii, :],
                                             start=(kc8 == 0), stop=(kc8 == 7))
                        i0 = ib * IBLK + g4 * 4
                        t, r = i0 // 128, i0 % 128
                        sc4 = sc4p.tile([4, N], F32)
                        nc.vector.tensor_copy(out=sc4, in_=lt[0:4, 0:N])
                        nc.sync.dma_start(out=L_sb[r:r + 4, t, :], in_=sc4)

            if _debug:
                for t, (toff, tsz) in enumerate(TCH):
                    nc.sync.dma_start(out=DBG["dbg_L"][toff:toff + tsz, :], in_=L_sb[0:tsz, t, :])
            # ================= symmetrize + logits + P0 + Sinkhorn =================
            with tc.tile_pool(name="psC", bufs=2, space="PSUM") as psC, \
                 tc.tile_pool(name="psV", bufs=2, space="PSUM") as psV:
                psL = psC.tile([128, 2, 256], F32, tag="ps")
                for t_out, (joff, jsz) in enumerate(TCH):
                    for t_in, (ioff, isz) in enumerate(TCH):
                        nc.tensor.transpose(psL[0:jsz, t_out, ioff:ioff + isz],
                                            L_sb[0:isz, t_in, joff:joff + jsz],
                                            ident[0:isz, 0:isz])
                Lsym = sb.tile([128, 2, N], F32)
                nc.vector.tensor_tensor(out=Lsym, in0=L_sb, in1=psL[:, :, 0:N], op=ALU.add)
                Lmh = sb.tile([128, 2, N], F32)
                nc.vector.tensor_tensor(out=Lmh, in0=Lsym, in1=maskh_sb, op=ALU.mult)
                lo = sb.tile([128, 2, N], F32)
                nc.vector.tensor_tensor(out=lo, in0=Lmh, in1=diagm_sb, op=ALU.add)
                for t, (toff, tsz) in enumerate(TCH):
                    nc.sync.dma_start(out=logits_d[toff:toff + tsz, :], in_=lo[0:tsz, t, :])

                mrow = sb.tile([128, 2], F32)
                for t, (toff, tsz) in enumerate(TCH):
                    nc.vector.reduce_max(out=mrow[0:tsz, t:t + 1], in_=Lmh[0:tsz, t, :],
                                         axis=AX.X)
                pmr = psV.tile([1, 2, 128], F32, tag="pv")
                for t, (toff, tsz) in enumerate(TCH):
                    nc.tensor.transpose(pmr[0:1, t, 0:tsz], mrow[0:tsz, t:t + 1],
                                        ident[0:tsz, 0:tsz])
                mjoin = sb.tile([1, N], F32)
                nc.vector.tensor_copy(out=mjoin[0:1, 0:128], in_=pmr[0:1, 0, :])
                nc.vector.tensor_copy(out=mjoin[0:1, 128:192], in_=pmr[0:1, 1, 0:64])
                gmax = sb.tile([1, 1], F32)
                nc.vector.reduce_max(out=gmax, in_=mjoin, axis=AX.X)
                pgb = psV.tile([128, 1], F32, tag="pg")
                nc.tensor.matmul(pgb, ones_row[0:1, 0:128], gmax, start=True, stop=True)
                negb = sb.tile([128, 1], F32)
                nc.vector.tensor_scalar(out=negb, in0=pgb, scalar1=-20.0, scalar2=None,
                                        op0=ALU.mult)

                P0 = sb.tile([128, 2, N], F32)
                nc.scalar.activation(out=P0, in_=lo, func=AF.Exp, bias=negb, scale=20.0)

                ccol0 = sb.tile([128, 2], F32)
                nc.vector.memset(ccol0, 1.0)
                cur = ccol0
                rcol = ccol = None
                for it in range(SINK_ITERS):
                    for half in range(2):
                        pd = psV.tile([128, 2], F32, tag="pd")
                        for mc, (moff, msz) in enumerate(TCH):
                            for kc, (koff, ksz) in enumerate(TCH):
                                nc.tensor.matmul(pd[0:msz, mc:mc + 1],
                                                 P0[0:ksz, kc, moff:moff + msz],
                                                 cur[0:ksz, kc:kc + 1],
                                                 start=(kc == 0), stop=(kc == 1))
                        nxt = sb.tile([128, 2], F32, tag=f"rc{half}{it % 2}")
                        nc.vector.reciprocal(out=nxt, in_=pd)
                        if half == 0:
                            rcol = nxt
                        else:
                            ccol = nxt
                        cur = nxt

                prr = psV.tile([1, 2, 128], F32, tag="pv")
                pcc = psV.tile([1, 2, 128], F32, tag="pv")
                for t, (toff, tsz) in enumerate(TCH):
                    nc.tensor.transpose(prr[0:1, t, 0:tsz], rcol[0:tsz, t:t + 1],
                                        ident[0:tsz, 0:tsz])
                    nc.tensor.transpose(pcc[0:1, t, 0:tsz], ccol[0:tsz, t:t + 1],
                                        ident[0:tsz, 0:tsz])
                rrow = sb.tile([1, N], F32)
                crow = sb.tile([1, N], F32)
                for t, (toff, tsz) in enumerate(TCH):
                    nc.vector.tensor_scalar(out=rrow[0:1, toff:toff + tsz],
                                            in0=prr[0:1, t, 0:tsz],
                                            scalar1=0.5, scalar2=None, op0=ALU.mult)
                    nc.vector.tensor_copy(out=crow[0:1, toff:toff + tsz],
                                          in_=pcc[0:1, t, 0:tsz])
                psS = psC.tile([128, 2, 256], F32, tag="ps")
                for mc, (moff, msz) in enumerate(TCH):
                    nc.tensor.matmul(psS[0:msz, mc, 0:N], rrow[0:1, moff:moff + msz], crow,
                                     start=True, stop=False)
                    nc.tensor.matmul(psS[0:msz, mc, 0:N], crow[0:1, moff:moff + msz], rrow,
                                     start=False, stop=True)
                x0sb = sb.tile([128, 2, N], F32)
                nc.vector.tensor_tensor(out=x0sb, in0=P0, in1=psS[:, :, 0:N], op=ALU.mult)
                for t, (toff, tsz) in enumerate(TCH):
                    nc.sync.dma_start(out=x0_d[toff:toff + tsz, :], in_=x0sb[0:tsz, t, :])

    nc.compile()
    return nc


_PROGRAM = None


def kernel(x, timesteps, params, _trace=False):
    global _PROGRAM
    in_maps = _host_prep(x, timesteps, params)
    if _PROGRAM is None:
        _PROGRAM = build_program()
    res = run_bass_kernel_spmd(_PROGRAM, in_maps, list(range(B)), trace=_trace)
    _PROFILE["last"] = res
    logits = np.stack([res.results[b]["logits"] for b in range(B)])
    x0 = np.stack([res.results[b]["x0"] for b in range(B)])
    return logits, x0


# revision 29
# speedup vs baseline: 1.1585x; 1.1585x over previous
"""Trainium2 Bass kernel for nn_DiscreteDiT (DiT backbone + pairwise MLP + Sinkhorn).

Self-contained: hardcodes all shapes. Shards batch B=8 across 8 NeuronCores
(one batch element per core, params replicated / host-prefolded per batch).
"""
import sys

for _p in ("/opt/trn_rl_repo", "/root/.axon_site/_ro/trn_rl_repo"):
    if _p not in sys.path:
        sys.path.insert(0, _p)

import numpy as np

import concourse.bacc as bacc
import concourse.mybir as mybir
import concourse.tile as tile
from concourse.bass_utils import run_bass_kernel_spmd
from concourse.masks import make_identity

F32 = mybir.dt.float32
F32R = mybir.dt.float32r
I32 = mybir.dt.int32
AF = mybir.ActivationFunctionType
ALU = mybir.AluOpType
AX = mybir.AxisListType

N = 192          # nodes / tokens
ENC = 1728       # encoding dim
D = 256          # d_model
LAYERS = 4
HEADS = 4
HD = 64
PROJ = 256
MLP_H = 256      # pair MLP hidden
BB_H = 1024      # backbone MLP hidden
SINK_ITERS = 20
B = 8

TCH = [(0, 128), (128, 64)]            # token chunks (start, size)
ECH = [(i * 128, 128) for i in range(13)] + [(1664, 64)]   # ENC chunks
NEC = len(ECH)
IBLK = 16                               # pair-stage i block

_PROFILE = {}


# ----------------------------------------------------------------- host prep
def _silu(x):
    return x / (1.0 + np.exp(-x))


def _np(a):
    return np.asarray(a, dtype=np.float64)


def _host_prep(x, timesteps, params):
    """Returns list of 8 per-core input dicts."""
    x = np.asarray(x, np.float64)
    ts = np.asarray(timesteps).astype(np.float64)
    p = params

    half = D // 2
    # match reference: freqs computed in fp32, args/sin/cos then in fp64 for accuracy
    freqs = np.exp(-np.log(np.float32(10000.0)) * np.arange(half, dtype=np.float32)
                   / np.float32(half)).astype(np.float32).astype(np.float64)
    args = ts[:, None] * freqs[None, :]
    temb = np.concatenate([np.sin(args), np.cos(args)], axis=-1)
    t = _silu(temb @ _np(p["t_mlp1"]["w"]) + _np(p["t_mlp1"]["b"]))
    t = t @ _np(p["t_mlp2"]["w"]) + _np(p["t_mlp2"]["b"])         # [B, D]
    st = _silu(t)

    w_in = _np(p["in_proj"]["w"])                                  # [ENC, D]
    pos_full = _np(p["pos_emb"]) + _np(p["in_proj"]["b"])[None, :]  # [N, D]

    fsmod = st @ _np(p["final_mod"]["w"]) + _np(p["final_mod"]["b"])  # [B, 2D]
    fs, fsc = fsmod[:, :D], fsmod[:, D:]

    w_f = _np(p["final"]["w"])
    b_f = _np(p["final"]["b"])

    w1 = _np(p["pair"][0]["w"])     # [2*PROJ, MLP_H]
    b1 = _np(p["pair"][0]["b"])
    w1a, w1b = w1[:PROJ], w1[PROJ:]
    w2p = _np(p["pair"][1]["w"])    # [MLP_H, MLP_H]
    b2p = _np(p["pair"][1]["b"])
    w3 = _np(p["pair_out"]["w"]).reshape(-1)   # [MLP_H]
    b3 = float(np.asarray(p["pair_out"]["b"]).reshape(-1)[0])

    w3blk = np.zeros((128, 8, 4), np.float64)
    for ii in range(4):
        for mc in range(2):
            w3blk[:, ii * 2 + mc, ii] = w3[mc * 128:(mc + 1) * 128]

    eye = np.eye(N, dtype=np.float64)
    mask_half = 0.5 * (1.0 - eye)
    diagm = b3 * (1.0 - eye) - 1e9 * eye

    lw = []
    for l in range(LAYERS):
        lp = p["layers"][l]
        mod = st @ _np(lp["mod"]["w"]) + _np(lp["mod"]["b"])       # [B, 6D]
        s1, sc1, g1, s2, sc2, g2 = np.split(mod, 6, axis=-1)
        lw.append(dict(
            wq=_np(lp["wq"]["w"]), bq=_np(lp["wq"]["b"]),
            wk=_np(lp["wk"]["w"]), bk=_np(lp["wk"]["b"]),
            wv=_np(lp["wv"]["w"]), bv=_np(lp["wv"]["b"]),
            wo=_np(lp["wo"]["w"]), bo=_np(lp["wo"]["b"]),
            w1=_np(lp["mlp1"]["w"]), b1=_np(lp["mlp1"]["b"]),
            w2=_np(lp["mlp2"]["w"]), b2=_np(lp["mlp2"]["b"]),
            s1=s1, sc1=sc1, g1=g1, s2=s2, sc2=sc2, g2=g2,
        ))

    in_maps = []
    sc = np.float32(1.0 / np.sqrt(HD))
    for b in range(B):
        m = {
            "xt": x[b].T,                                    # [ENC, N]
            "w_in": w_in,
            "pos": pos_full,
            "wf": (1.0 + fsc[b])[:, None] * w_f,
            "cf": fs[b] @ w_f + b_f,
            "w1a": w1a, "w1b": w1b, "cu": b1,
            "w2p": w2p, "b2p": b2p, "w3blk": w3blk,
            "mask_half": mask_half, "diagm": diagm,
            "ones_r": np.ones(2 * N, np.float32),
            "b2p_r": np.asarray(b2p, np.float32),
        }
        for l, W in enumerate(lw):
            d1 = (1.0 + W["sc1"][b])[:, None]
            d2 = (1.0 + W["sc2"][b])[:, None]
            m[f"wq{l}"] = d1 * W["wq"] * sc
            m[f"cq{l}"] = (W["s1"][b] @ W["wq"] + W["bq"]) * sc
            m[f"wk{l}"] = d1 * W["wk"]
            m[f"ck{l}"] = W["s1"][b] @ W["wk"] + W["bk"]
            m[f"wv{l}"] = d1 * W["wv"]
            m[f"cv{l}"] = W["s1"][b] @ W["wv"] + W["bv"]
            m[f"wo{l}"] = W["wo"] * W["g1"][b][None, :]
            m[f"co{l}"] = W["bo"] * W["g1"][b]
            m[f"w1{l}"] = d2 * W["w1"]
            m[f"c1{l}"] = W["s2"][b] @ W["w1"] + W["b1"]
            m[f"w2{l}"] = W["w2"] * W["g2"][b][None, :]
            m[f"c2{l}"] = W["b2"] * W["g2"][b]
        m = {k: np.ascontiguousarray(v, dtype=np.float32) for k, v in m.items()}
        in_maps.append(m)
    return in_maps


# ------------------------------------------------------------- device program
def build_program(_debug=False):
    nc = bacc.Bacc("TRN2", target_bir_lowering=False, debug=False)

    def din(name, shape):
        return nc.dram_tensor(name, list(shape), F32, kind="ExternalInput").ap()

    xt_d = nc.dram_tensor("xt", [ENC, N], F32R, kind="ExternalInput").ap()
    win_d = nc.dram_tensor("w_in", [ENC, D], F32R, kind="ExternalInput").ap()
    pos_d = din("pos", (N, D))
    wf_d = nc.dram_tensor("wf", [D, D], F32R, kind="ExternalInput").ap()
    cf_d = din("cf", (D,))
    w1a_d = nc.dram_tensor("w1a", [PROJ, MLP_H], F32R, kind="ExternalInput").ap()
    w1b_d = nc.dram_tensor("w1b", [PROJ, MLP_H], F32R, kind="ExternalInput").ap()
    cu_d = din("cu", (MLP_H,))
    w2p_d = nc.dram_tensor("w2p", [MLP_H, MLP_H], F32R, kind="ExternalInput").ap()
    b2p_d = din("b2p", (MLP_H,))
    w3blk_d = nc.dram_tensor("w3blk", [128, 8, 4], F32R, kind="ExternalInput").ap()
    onesr_d = nc.dram_tensor("ones_r", [2 * N], F32R, kind="ExternalInput").ap()
    b2pr_d = nc.dram_tensor("b2p_r", [MLP_H], F32R, kind="ExternalInput").ap()
    maskh_d = din("mask_half", (N, N))
    diagm_d = din("diagm", (N, N))
    LW = []
    for l in range(LAYERS):
        ent = {}
        for k, shp in [("wq", (D, D)), ("cq", (D,)), ("wk", (D, D)), ("ck", (D,)),
                       ("wv", (D, D)), ("cv", (D,)), ("wo", (D, D)), ("co", (D,)),
                       ("w1", (D, BB_H)), ("c1", (BB_H,)), ("w2", (BB_H, D)), ("c2", (D,))]:
            dt_ = F32R if k in ("wq", "wk", "w1", "wv", "wo", "w2") else F32
            ent[k] = nc.dram_tensor(f"{k}{l}", list(shp), dt_, kind="ExternalInput").ap()
        LW.append(ent)

    logits_d = nc.dram_tensor("logits", [N, N], F32, kind="ExternalOutput").ap()
    x0_d = nc.dram_tensor("x0", [N, N], F32, kind="ExternalOutput").ap()
    DBG = {}
    if _debug:
        for nm, shp in [("dbg_h0", (N, D)), ("dbg_h1", (N, D)), ("dbg_h2", (N, D)),
                        ("dbg_h3", (N, D)), ("dbg_h4", (N, D)),
                        ("dbg_hfT", (D, N)), ("dbg_uT", (D, N)), ("dbg_vT", (D, N)),
                        ("dbg_L", (N, N))]:
            DBG[nm] = nc.dram_tensor(nm, list(shp), F32, kind="ExternalOutput").ap()

    def dump_h(nm):
        if _debug:
            for t, (toff, tsz) in enumerate(TCH):
                nc.sync.dma_start(out=DBG[nm][toff:toff + tsz, :], in_=h_sb[0:tsz, t, :])

    def dump_T(nm, tl):
        if _debug:
            for c in range(2):
                nc.sync.dma_start(out=DBG[nm][c * 128:(c + 1) * 128, :], in_=tl[:, c, :])

    col2 = lambda a: a.rearrange("(c p) -> p c", p=128)     # [2X*128] -> [128, 2X]
    row = lambda a: a.rearrange("(o p) -> o p", o=1)        # [X] -> [1, X]

    with tile.TileContext(nc) as tc:
        with nc.allow_low_precision(reason="f32r intermediates; validated end-to-end"), \
             tc.tile_pool(name="cons", bufs=1) as cons, \
             tc.tile_pool(name="sb", bufs=1) as sb:

            ident = cons.tile([128, 128], F32)
            make_identity(nc, ident)
            ones_row = cons.tile([1, 192], F32)
            nc.vector.memset(ones_row, 1.0)
            ones_row_r = cons.tile([1, 2 * N], F32R)
            nc.sync.dma_start(out=ones_row_r, in_=onesr_d.rearrange("(o p) -> o p", o=1))
            ones_col = cons.tile([128, 1], F32)
            nc.vector.memset(ones_col, 1.0)
            epst = cons.tile([128, 1], F32)
            nc.vector.memset(epst, 1e-6)

            h_sb = cons.tile([128, 2, D], F32)

            def layer_norm(wp, psp, use_newton=False):
                """LN of h_sb -> transposed hlnT [128, 2kc, N]."""
                mv = []
                for t, (toff, tsz) in enumerate(TCH):
                    st_t = wp.tile([128, 6], F32, tag="bnst")
                    nc.vector.bn_stats(out=st_t[0:tsz, :], in_=h_sb[0:tsz, t, :])
                    mv_t = wp.tile([128, 2], F32, tag=f"bnmv{t}")
                    nc.vector.bn_aggr(out=mv_t[0:tsz, :], in_=st_t[0:tsz, :])
                    mv.append(mv_t)
                vpack = wp.tile([128, 2], F32, tag="vpack")
                nc.vector.tensor_copy(out=vpack[:, 0:1], in_=mv[0][:, 1:2])
                nc.vector.tensor_copy(out=vpack[0:64, 1:2], in_=mv[1][0:64, 1:2])
                rstd = wp.tile([128, 2], F32, tag="rstd")
                if not use_newton:
                    lnv = wp.tile([128, 2], F32, tag="lnv")
                    nc.scalar.activation(out=lnv, in_=vpack, func=AF.Ln, bias=epst, scale=1.0)
                    nc.scalar.activation(out=rstd, in_=lnv, func=AF.Exp, bias=0.0, scale=-0.5)
                else:
                    a = wp.tile([128, 2], F32, tag="nta")
                    nc.vector.tensor_scalar(out=a, in0=vpack, scalar1=1e-6, scalar2=None,
                                            op0=ALU.add)
                    xh = wp.tile([128, 2], F32, tag="ntxh")
                    nc.vector.tensor_scalar(out=xh, in0=a, scalar1=0.5, scalar2=None,
                                            op0=ALU.mult)
                    ti = wp.tile([128, 2], I32, tag="nti")
                    nc.vector.tensor_scalar(out=ti, in0=a.bitcast(I32), scalar1=1,
                                            scalar2=None, op0=ALU.logical_shift_right)
                    nc.vector.tensor_scalar(out=ti, in0=ti, scalar1=-1, scalar2=None,
                                            op0=ALU.bitwise_xor)
                    nc.vector.tensor_scalar(out=ti, in0=ti, scalar1=0x5f3759df + 1,
                                            scalar2=None, op0=ALU.add)
                    y = rstd
                    nc.vector.tensor_copy(out=y, in_=ti.bitcast(F32))
                    t1 = wp.tile([128, 2], F32, tag="ntt1")
                    for _ in range(3):
                        nc.vector.tensor_tensor(out=t1, in0=y, in1=y, op=ALU.mult)
                        nc.vector.tensor_tensor(out=t1, in0=t1, in1=xh, op=ALU.mult)
                        nc.vector.tensor_scalar(out=t1, in0=t1, scalar1=-1.0, scalar2=1.5,
                                                op0=ALU.mult, op1=ALU.add)
                        nc.vector.tensor_tensor(out=y, in0=y, in1=t1, op=ALU.mult)
                hln = wp.tile([128, 2, D], F32, tag="hln")
                for t, (toff, tsz) in enumerate(TCH):
                    nc.vector.tensor_scalar(out=hln[0:tsz, t, :], in0=h_sb[0:tsz, t, :],
                                            scalar1=mv[t][0:tsz, 0:1],
                                            scalar2=rstd[0:tsz, t:t + 1],
                                            op0=ALU.subtract, op1=ALU.mult)
                pt = psp.tile([128, 2, 256], F32, tag="ps")
                for kc in range(2):
                    for t, (toff, tsz) in enumerate(TCH):
                        nc.tensor.transpose(pt[0:128, kc, toff:toff + tsz],
                                            hln[0:tsz, t, kc * 128:(kc + 1) * 128],
                                            ident[0:tsz, 0:tsz])
                hlnT = wp.tile([128, 2, 256], F32R, tag="hlnT")
                nc.vector.tensor_copy(out=hlnT[:, :, 0:N], in_=pt[:, :, 0:N])
                return hlnT

            # ================= in_proj =================
            with tc.tile_pool(name="inp", bufs=1) as inp, \
                 tc.tile_pool(name="psA", bufs=2, space="PSUM") as psA:
                xt_sb = inp.tile([128, NEC, N], F32R)
                win_sb = inp.tile([128, NEC, D], F32R)
                pos_sb = inp.tile([128, 2, D], F32)
                for c, (off, sz) in enumerate(ECH):
                    nc.sync.dma_start(out=xt_sb[0:sz, c, :], in_=xt_d[off:off + sz, :])
                    nc.sync.dma_start(out=win_sb[0:sz, c, :], in_=win_d[off:off + sz, :])
                for t, (toff, tsz) in enumerate(TCH):
                    nc.sync.dma_start(out=pos_sb[0:tsz, t, :], in_=pos_d[toff:toff + tsz, :])
                for t, (toff, tsz) in enumerate(TCH):
                    ph = psA.tile([128, D], F32, tag="ph")
                    for c, (off, sz) in enumerate(ECH):
                        if tsz == 128:
                            nc.tensor.matmul(ph[0:tsz, :], xt_sb[0:sz, c, toff:toff + tsz],
                                             win_sb[0:sz, c, :],
                                             start=(c == 0), stop=(c == NEC - 1))
                        else:
                            nc.tensor.matmul(ph[0:tsz, :],
                                             xt_sb.bitcast(F32)[0:sz, c, toff:toff + tsz],
                                             win_sb.bitcast(F32)[0:sz, c, :],
                                             start=(c == 0), stop=(c == NEC - 1))
                    nc.vector.tensor_tensor(out=h_sb[0:tsz, t, :], in0=ph[0:tsz, :],
                                            in1=pos_sb[0:tsz, t, :], op=ALU.add)
                dump_h("dbg_h0")

            # ================= backbone =================
            with tc.tile_pool(name="wpool", bufs=2) as wpool, \
                 tc.tile_pool(name="tpool", bufs=2) as tpool, \
                 tc.tile_pool(name="psB", bufs=6, space="PSUM") as psB, \
                 tc.tile_pool(name="psS", bufs=2, space="PSUM") as psSm:
                for l in range(LAYERS):
                    wd = LW[l]
                    wq = wpool.tile([128, 2, D], F32R, tag="wq")
                    wk = wpool.tile([128, 2, D], F32R, tag="wk")
                    wv = wpool.tile([128, 2, D], F32R, tag="wv")
                    wo = wpool.tile([128, 2, D], F32R, tag="wo")
                    w1 = wpool.tile([128, 2, BB_H], F32R, tag="w1")
                    w2 = wpool.tile([128, 8, D], F32R, tag="w2")
                    for w_sb, w_dr in [(wq, wd["wq"]), (wk, wd["wk"]), (wv, wd["wv"]),
                                       (wo, wd["wo"]), (w1, wd["w1"]), (w2, wd["w2"])]:
                        nc.sync.dma_start(out=w_sb, in_=w_dr.rearrange("(c p) m -> p c m", p=128))
                    cq = wpool.tile([128, 2], F32, tag="cq")
                    nc.sync.dma_start(out=cq, in_=col2(wd["cq"]))
                    ck = wpool.tile([128, 2], F32, tag="ck")
                    nc.sync.dma_start(out=ck, in_=col2(wd["ck"]))
                    c1 = wpool.tile([128, 8], F32, tag="c1")
                    nc.sync.dma_start(out=c1, in_=col2(wd["c1"]))
                    cv = wpool.tile([1, D], F32, tag="cv")
                    nc.sync.dma_start(out=cv, in_=row(wd["cv"]))
                    co = wpool.tile([1, D], F32, tag="co")
                    nc.sync.dma_start(out=co, in_=row(wd["co"]))
                    c2 = wpool.tile([1, D], F32, tag="c2")
                    nc.sync.dma_start(out=c2, in_=row(wd["c2"]))

                    # --- attention ---
                    hlnT = layer_norm(tpool, psB)
                    qT = tpool.tile([128, 2, N], F32, tag="qT")
                    kT = tpool.tile([128, 2, N], F32, tag="kT")
                    for w_sb, c_sb, out_sb in [(wq, cq, qT), (wk, ck, kT)]:
                        pq = psB.tile([128, 2, 256], F32, tag="ps")
                        for mc in range(2):
                            for kc in range(2):
                                nc.tensor.matmul(pq[:, mc, :],
                                                 w_sb[:, kc, mc * 128:(mc + 1) * 128],
                                                 hlnT[:, kc, :], start=(kc == 0), stop=(kc == 1))
                        for mc in range(2):
                            nc.vector.tensor_scalar(out=out_sb[:, mc, :], in0=pq[:, mc, 0:N],
                                                    scalar1=c_sb[:, mc:mc + 1], scalar2=None,
                                                    op0=ALU.add)
                    pv = psB.tile([128, 2, 256], F32, tag="ps")
                    for t, (toff, tsz) in enumerate(TCH):
                        for kc in range(2):
                            if tsz == 128:
                                nc.tensor.matmul(pv[0:tsz, t, :],
                                                 hlnT[:, kc, toff:toff + tsz],
                                                 wv[:, kc, :], start=(kc == 0), stop=False)
                            else:
                                nc.tensor.matmul(pv[0:tsz, t, :],
                                                 hlnT.bitcast(F32)[:, kc, toff:toff + tsz],
                                                 wv.bitcast(F32)[:, kc, :],
                                                 start=(kc == 0), stop=False)
                        nc.tensor.matmul(pv[0:tsz, t, :], ones_row[0:1, 0:tsz], cv,
                                         start=False, stop=True)
                    v_sb = tpool.tile([128, 2, D], F32, tag="v_sb")
                    nc.vector.tensor_copy(out=v_sb, in_=pv)

                    ET = tpool.tile([128, 2, HEADS, N], F32, tag="ET")
                    for hh in range(HEADS):
                        hp, mc = 64 * (hh % 2), hh // 2
                        pss = psB.tile([128, 2, 256], F32, tag="ps")
                        for t2, (t2off, t2sz) in enumerate(TCH):
                            nc.tensor.matmul(pss[0:t2sz, t2, 0:N],
                                             kT[hp:hp + 64, mc, t2off:t2off + t2sz],
                                             qT[hp:hp + 64, mc, :], start=True, stop=True)
                        nc.scalar.activation(out=ET[:, :, hh, :], in_=pss[:, :, 0:N],
                                             func=AF.Exp, bias=0.0, scale=1.0)
                    for hpair in range(2):
                        prs = psSm.tile([1, 2, 256], F32, tag="pr")
                        for hi in range(2):
                            hh = hpair * 2 + hi
                            for t2, (t2off, t2sz) in enumerate(TCH):
                                nc.tensor.matmul(prs[0:1, hi, 0:N], ones_col[0:t2sz, 0:1],
                                                 ET[0:t2sz, t2, hh, :],
                                                 start=(t2 == 0), stop=(t2 == 1))
                        rsr = tpool.tile([1, 2, N], F32, tag="rsr")
                        nc.vector.reciprocal(out=rsr, in_=prs[0:1, :, 0:N])
                        pbc = psB.tile([128, 2, 256], F32, tag="ps")
                        for hi in range(2):
                            nc.tensor.matmul(pbc[:, hi, 0:N], ones_row[0:1, 0:128],
                                             rsr[0:1, hi, :], start=True, stop=True)
                        for t2 in range(2):
                            nc.vector.tensor_tensor(
                                out=ET[:, t2, hpair * 2:hpair * 2 + 2, :],
                                in0=ET[:, t2, hpair * 2:hpair * 2 + 2, :],
                                in1=pbc[:, :, 0:N], op=ALU.mult)
                    po = psB.tile([128, 2, 256], F32, tag="ps")
                    for hh in range(HEADS):
                        hp, s = 64 * (hh % 2), hh // 2
                        for t2, (t2off, t2sz) in enumerate(TCH):
                            nc.tensor.matmul(po[hp:hp + 64, s, 0:N],
                                             v_sb[0:t2sz, t2, hh * 64:hh * 64 + 64],
                                             ET[0:t2sz, t2, hh, :],
                                             start=(t2 == 0), stop=(t2 == 1),
                                             tile_position=(0, hp))
                    oT = tpool.tile([128, 2, N], F32R, tag="oT")
                    nc.vector.tensor_copy(out=oT, in_=po[:, :, 0:N])
                    pa = psB.tile([128, 2, 256], F32, tag="ps")
                    for t, (toff, tsz) in enumerate(TCH):
                        for kc in range(2):
                            if tsz == 128:
                                nc.tensor.matmul(pa[0:tsz, t, :], oT[:, kc, toff:toff + tsz],
                                                 wo[:, kc, :], start=(kc == 0), stop=False)
                            else:
                                nc.tensor.matmul(pa[0:tsz, t, :],
                                                 oT.bitcast(F32)[:, kc, toff:toff + tsz],
                                                 wo.bitcast(F32)[:, kc, :],
                                                 start=(kc == 0), stop=False)
                        nc.tensor.matmul(pa[0:tsz, t, :], ones_row[0:1, 0:tsz], co,
                                         start=False, stop=True)
                    for t, (toff, tsz) in enumerate(TCH):
                        nc.vector.tensor_tensor(out=h_sb[0:tsz, t, :], in0=h_sb[0:tsz, t, :],
                                                in1=pa[0:tsz, t, :], op=ALU.add)

                    # --- mlp ---
                    hln2T = layer_norm(tpool, psB)
                    gm1T = tpool.tile([128, 8, N], F32R, tag="gm1T")
                    for mc in range(8):
                        pm = psB.tile([128, 256], F32, tag="ps")
                        for kc in range(2):
                            nc.tensor.matmul(pm[:, :],
                                             w1[:, kc, mc * 128:(mc + 1) * 128],
                                             hln2T[:, kc, :],
                                             start=(kc == 0), stop=(kc == 1))
                        nc.scalar.activation(out=gm1T[:, mc, :], in_=pm[:, 0:N],
                                             func=AF.Gelu_apprx_tanh,
                                             bias=c1[:, mc:mc + 1], scale=1.0)
                    pa2 = psB.tile([128, 2, 256], F32, tag="ps")
                    for t, (toff, tsz) in enumerate(TCH):
                        for kc in range(8):
                            if tsz == 128:
                                nc.tensor.matmul(pa2[0:tsz, t, :], gm1T[:, kc, toff:toff + tsz],
                                                 w2[:, kc, :], start=(kc == 0), stop=False)
                            else:
                                nc.tensor.matmul(pa2[0:tsz, t, :],
                                                 gm1T.bitcast(F32)[:, kc, toff:toff + tsz],
                                                 w2.bitcast(F32)[:, kc, :],
                                                 start=(kc == 0), stop=False)
                        nc.tensor.matmul(pa2[0:tsz, t, :], ones_row[0:1, 0:tsz], c2,
                                         start=False, stop=True)
                    for t, (toff, tsz) in enumerate(TCH):
                        nc.vector.tensor_tensor(out=h_sb[0:tsz, t, :], in0=h_sb[0:tsz, t, :],
                                                in1=pa2[0:tsz, t, :], op=ALU.add)
                    dump_h(f"dbg_h{l + 1}")

                # ============ final LN + projections (inside backbone pools) ============
                wf_sb = sb.tile([128, 2, D], F32R)
                nc.sync.dma_start(out=wf_sb, in_=wf_d.rearrange("(c p) m -> p c m", p=128))
                cf_sb = sb.tile([128, 2], F32)
                nc.sync.dma_start(out=cf_sb, in_=col2(cf_d))
                w1a_sb = sb.tile([128, 2, MLP_H], F32R)
                nc.sync.dma_start(out=w1a_sb, in_=w1a_d.rearrange("(c p) m -> p c m", p=128))
                w1b_sb = sb.tile([128, 2, MLP_H], F32R)
                nc.sync.dma_start(out=w1b_sb, in_=w1b_d.rearrange("(c p) m -> p c m", p=128))
                cu_sb = sb.tile([128, 2], F32)
                nc.sync.dma_start(out=cu_sb, in_=col2(cu_d))

                hlnfT = layer_norm(tpool, psB, use_newton=True)
                uT = sb.tile([128, 2, N], F32)
                vT = sb.tile([128, 2, N], F32)
                hfT = sb.tile([128, 2, 256], F32R)
                pf = psB.tile([128, 2, 256], F32, tag="ps")
                for mc in range(2):
                    for kc in range(2):
                        nc.tensor.matmul(pf[:, mc, :], wf_sb[:, kc, mc * 128:(mc + 1) * 128],
                                         hlnfT[:, kc, :], start=(kc == 0), stop=(kc == 1))
                for mc in range(2):
                    nc.vector.tensor_scalar(out=hfT[:, mc, 0:N], in0=pf[:, mc, 0:N],
                                            scalar1=cf_sb[:, mc:mc + 1], scalar2=None,
                                            op0=ALU.add)
                pu = psB.tile([128, 2, 256], F32, tag="ps")
                pvv = psB.tile([128, 2, 256], F32, tag="ps")
                for mc in range(2):
                    for kc in range(2):
                        nc.tensor.matmul(pu[:, mc, :], w1a_sb[:, kc, mc * 128:(mc + 1) * 128],
                                         hfT[:, kc, :], start=(kc == 0), stop=(kc == 1))
                        nc.tensor.matmul(pvv[:, mc, :], w1b_sb[:, kc, mc * 128:(mc + 1) * 128],
                                         hfT[:, kc, :], start=(kc == 0), stop=(kc == 1))
                for mc in range(2):
                    nc.vector.tensor_scalar(out=uT[:, mc, :], in0=pu[:, mc, 0:N],
                                            scalar1=cu_sb[:, mc:mc + 1], scalar2=None,
                                            op0=ALU.add)
                nc.vector.tensor_copy(out=vT, in_=pvv[:, :, 0:N])
                dump_T("dbg_hfT", hfT)
                dump_T("dbg_uT", uT)
                dump_T("dbg_vT", vT)

            # ================= pairwise MLP =================
            w2p_sb = sb.tile([128, 2, MLP_H], F32R)
            nc.sync.dma_start(out=w2p_sb, in_=w2p_d.rearrange("(c p) m -> p c m", p=128))
            b2p_sb = sb.tile([128, 2], F32)
            nc.sync.dma_start(out=b2p_sb, in_=col2(b2p_d))
            b2pr_sb = sb.tile([1, MLP_H], F32R)
            nc.sync.dma_start(out=b2pr_sb, in_=b2pr_d.rearrange("(o p) -> o p", o=1))
            w3b_sb = sb.tile([128, 8, 4], F32R)
            nc.sync.dma_start(out=w3b_sb, in_=w3blk_d)
            maskh_sb = sb.tile([128, 2, N], F32)
            diagm_sb = sb.tile([128, 2, N], F32)
            for t, (toff, tsz) in enumerate(TCH):
                nc.sync.dma_start(out=maskh_sb[0:tsz, t, :], in_=maskh_d[toff:toff + tsz, :])
                nc.sync.dma_start(out=diagm_sb[0:tsz, t, :], in_=diagm_d[toff:toff + tsz, :])

            L_sb = sb.tile([128, 2, N], F32)
            with tc.tile_pool(name="zp", bufs=2) as zp, \
                 tc.tile_pool(name="gzp", bufs=2) as gzp, \
                 tc.tile_pool(name="g2p", bufs=3) as g2p, \
                 tc.tile_pool(name="sc4", bufs=3) as sc4p, \
                 tc.tile_pool(name="pps", bufs=3, space="PSUM") as pps, \
                 tc.tile_pool(name="plt", bufs=2, space="PSUM") as plt:
                for ib in range(N // IBLK):
                    z = zp.tile([128, 2, IBLK * N], F32)
                    gz = gzp.tile([128, 2, IBLK * N], F32R)
                    for ii in range(IBLK):
                        i = ib * IBLK + ii
                        for kc in range(2):
                            nc.vector.tensor_scalar(out=z[:, kc, ii * N:(ii + 1) * N],
                                                    in0=vT[:, kc, :],
                                                    scalar1=uT[:, kc, i:i + 1],
                                                    scalar2=None, op0=ALU.add)
                    for kc in range(2):
                        nc.scalar.activation(out=gz[:, kc, :], in_=z[:, kc, :],
                                             func=AF.Gelu_apprx_tanh, bias=0.0, scale=1.0)
                    for g4 in range(IBLK // 4):
                        g2sb = g2p.tile([128, 2, 4, 256], F32R)
                        for pp in range(2):
                            # [128, mc, 512-pad]: one accumulation group per bank,
                            # N=384 covers two consecutive i's in one matmul
                            ps2 = pps.tile([128, 2, 512], F32)
                            iioff = (g4 * 4 + pp * 2) * N
                            for mc in range(2):
                                for kc in range(2):
                                    nc.tensor.matmul(
                                        ps2[:, mc, 0:2 * N],
                                        w2p_sb[:, kc, mc * 128:(mc + 1) * 128],
                                        gz[:, kc, iioff:iioff + 2 * N],
                                        start=(kc == 0), stop=False)
                                nc.tensor.matmul(
                                    ps2[:, mc, 0:2 * N],
                                    b2pr_sb[0:1, mc * 128:(mc + 1) * 128],
                                    ones_row_r[0:1, 0:2 * N],
                                    start=False, stop=True)
                            nc.scalar.activation(
                                out=g2sb[:, :, pp * 2:pp * 2 + 2, 0:N],
                                in_=ps2[:, :, 0:2 * N].rearrange(
                                    "p a (b n) -> p a b n", n=N),
                                func=AF.Gelu_apprx_tanh, bias=0.0, scale=1.0)
                        lt = plt.tile([4, 256], F32)
                        for kc8 in range(8):
                            ii, mc = kc8 // 2, kc8 % 2
                            nc.tensor.matmul(lt[0:4, :], w3b_sb[:, kc8, :],
                                             g2sb[:, mc, ii, :],
                                             start=(kc8 == 0), stop=(kc8 == 7))
                        i0 = ib * IBLK + g4 * 4
                        t, r = i0 // 128, i0 % 128
                        sc4 = sc4p.tile([4, N], F32)
                        nc.vector.tensor_copy(out=sc4, in_=lt[0:4, 0:N])
                        nc.sync.dma_start(out=L_sb[r:r + 4, t, :], in_=sc4)

            if _debug:
                for t, (toff, tsz) in enumerate(TCH):
                    nc.sync.dma_start(out=DBG["dbg_L"][toff:toff + tsz, :], in_=L_sb[0:tsz, t, :])
            # ================= symmetrize + logits + P0 + Sinkhorn =================
            with tc.tile_pool(name="psC", bufs=2, space="PSUM") as psC, \
                 tc.tile_pool(name="psV", bufs=2, space="PSUM") as psV:
                psL = psC.tile([128, 2, 256], F32, tag="ps")
                for t_out, (joff, jsz) in enumerate(TCH):
                    for t_in, (ioff, isz) in enumerate(TCH):
                        nc.tensor.transpose(psL[0:jsz, t_out, ioff:ioff + isz],
                                            L_sb[0:isz, t_in, joff:joff + jsz],
                                            ident[0:isz, 0:isz])
                Lsym = sb.tile([128, 2, N], F32)
                nc.vector.tensor_tensor(out=Lsym, in0=L_sb, in1=psL[:, :, 0:N], op=ALU.add)
                Lmh = sb.tile([128, 2, N], F32)
                nc.vector.tensor_tensor(out=Lmh, in0=Lsym, in1=maskh_sb, op=ALU.mult)
                lo = sb.tile([128, 2, N], F32)
                nc.vector.tensor_tensor(out=lo, in0=Lmh, in1=diagm_sb, op=ALU.add)
                for t, (toff, tsz) in enumerate(TCH):
                    nc.sync.dma_start(out=logits_d[toff:toff + tsz, :], in_=lo[0:tsz, t, :])

                mrow = sb.tile([128, 2], F32)
                for t, (toff, tsz) in enumerate(TCH):
                    nc.vector.reduce_max(out=mrow[0:tsz, t:t + 1], in_=Lmh[0:tsz, t, :],
                                         axis=AX.X)
                pmr = psV.tile([1, 2, 128], F32, tag="pv")
                for t, (toff, tsz) in enumerate(TCH):
                    nc.tensor.transpose(pmr[0:1, t, 0:tsz], mrow[0:tsz, t:t + 1],
                                        ident[0:tsz, 0:tsz])
                mjoin = sb.tile([1, N], F32)
                nc.vector.tensor_copy(out=mjoin[0:1, 0:128], in_=pmr[0:1, 0, :])
                nc.vector.tensor_copy(out=mjoin[0:1, 128:192], in_=pmr[0:1, 1, 0:64])
                gmax = sb.tile([1, 1], F32)
                nc.vector.reduce_max(out=gmax, in_=mjoin, axis=AX.X)
                pgb = psV.tile([128, 1], F32, tag="pg")
                nc.tensor.matmul(pgb, ones_row[0:1, 0:128], gmax, start=True, stop=True)
                negb = sb.tile([128, 1], F32)
                nc.vector.tensor_scalar(out=negb, in0=pgb, scalar1=-20.0, scalar2=None,
                                        op0=ALU.mult)

                P0 = sb.tile([128, 2, N], F32)
                nc.scalar.activation(out=P0, in_=lo, func=AF.Exp, bias=negb, scale=20.0)

                ccol0 = sb.tile([128, 2], F32)
                nc.vector.memset(ccol0, 1.0)
                cur = ccol0
                rcol = ccol = None
                for it in range(SINK_ITERS):
                    for half in range(2):
                        pd = psV.tile([128, 2], F32, tag="pd")
                        for mc, (moff, msz) in enumerate(TCH):
                            for kc, (koff, ksz) in enumerate(TCH):
                                nc.tensor.matmul(pd[0:msz, mc:mc + 1],
                                                 P0[0:ksz, kc, moff:moff + msz],
                                                 cur[0:ksz, kc:kc + 1],
                                                 start=(kc == 0), stop=(kc == 1))
                        nxt = sb.tile([128, 2], F32, tag=f"rc{half}{it % 2}")
                        nc.vector.reciprocal(out=nxt, in_=pd)
                        if half == 0:
                            rcol = nxt
                        else:
                            ccol = nxt
                        cur = nxt

                prr = psV.tile([1, 2, 128], F32, tag="pv")
                pcc = psV.tile([1, 2, 128], F32, tag="pv")
                for t, (toff, tsz) in enumerate(TCH):
                    nc.tensor.transpose(prr[0:1, t, 0:tsz], rcol[0:tsz, t:t + 1],
                                        ident[0:tsz, 0:tsz])
                    nc.tensor.transpose(pcc[0:1, t, 0:tsz], ccol[0:tsz, t:t + 1],
                                        ident[0:tsz, 0:tsz])
                rrow = sb.tile([1, N], F32)
                crow = sb.tile([1, N], F32)
                for t, (toff, tsz) in enumerate(TCH):
                    nc.vector.tensor_scalar(out=rrow[0:1, toff:toff + tsz],
                                            in0=prr[0:1, t, 0:tsz],
                                            scalar1=0.5, scalar2=None, op0=ALU.mult)
                    nc.vector.tensor_copy(out=crow[0:1, toff:toff + tsz],
                                          in_=pcc[0:1, t, 0:tsz])
                psS = psC.tile([128, 2, 256], F32, tag="ps")
                for mc, (moff, msz) in enumerate(TCH):
                    nc.tensor.matmul(psS[0:msz, mc, 0:N], rrow[0:1, moff:moff + msz], crow,
                                     start=True, stop=False)
                    nc.tensor.matmul(psS[0:msz, mc, 0:N], crow[0:1, moff:moff + msz], rrow,
                                     start=False, stop=True)
                x0sb = sb.tile([128, 2, N], F32)
                nc.vector.tensor_tensor(out=x0sb, in0=P0, in1=psS[:, :, 0:N], op=ALU.mult)
                for t, (toff, tsz) in enumerate(TCH):
                    nc.sync.dma_start(out=x0_d[toff:toff + tsz, :], in_=x0sb[0:tsz, t, :])

    nc.compile()
    return nc


_PROGRAM = None


def kernel(x, timesteps, params, _trace=False):
    global _PROGRAM
    in_maps = _host_prep(x, timesteps, params)
    if _PROGRAM is None:
        _PROGRAM = build_program()
    res = run_bass_kernel_spmd(_PROGRAM, in_maps, list(range(B)), trace=_trace)
    _PROFILE["last"] = res
    logits = np.stack([res.results[b]["logits"] for b in range(B)])
    x0 = np.stack([res.results[b]["x0"] for b in range(B)])
    return logits, x0


# revision 32
# speedup vs baseline: 1.2060x; 1.0410x over previous
"""Trainium2 Bass kernel for nn_DiscreteDiT (DiT backbone + pairwise MLP + Sinkhorn).

Self-contained: hardcodes all shapes. Shards batch B=8 across 8 NeuronCores
(one batch element per core, params replicated / host-prefolded per batch).
"""
import sys

for _p in ("/opt/trn_rl_repo", "/root/.axon_site/_ro/trn_rl_repo"):
    if _p not in sys.path:
        sys.path.insert(0, _p)

import numpy as np

import concourse.bacc as bacc
import concourse.mybir as mybir
import concourse.tile as tile
from concourse.bass_utils import run_bass_kernel_spmd
from concourse.masks import make_identity

F32 = mybir.dt.float32
F32R = mybir.dt.float32r
I32 = mybir.dt.int32
AF = mybir.ActivationFunctionType
ALU = mybir.AluOpType
AX = mybir.AxisListType

N = 192          # nodes / tokens
ENC = 1728       # encoding dim
D = 256          # d_model
LAYERS = 4
HEADS = 4
HD = 64
PROJ = 256
MLP_H = 256      # pair MLP hidden
BB_H = 1024      # backbone MLP hidden
SINK_ITERS = 20
B = 8

TCH = [(0, 128), (128, 64)]            # token chunks (start, size)
ECH = [(i * 128, 128) for i in range(13)] + [(1664, 64)]   # ENC chunks
NEC = len(ECH)
IBLK = 16                               # pair-stage i block

_PROFILE = {}


# ----------------------------------------------------------------- host prep
def _silu(x):
    return x / (1.0 + np.exp(-x))


def _np(a):
    return np.asarray(a, dtype=np.float64)


def _host_prep(x, timesteps, params):
    """Returns list of 8 per-core input dicts."""
    x = np.asarray(x, np.float64)
    ts = np.asarray(timesteps).astype(np.float64)
    p = params

    half = D // 2
    # match reference: freqs computed in fp32, args/sin/cos then in fp64 for accuracy
    freqs = np.exp(-np.log(np.float32(10000.0)) * np.arange(half, dtype=np.float32)
                   / np.float32(half)).astype(np.float32).astype(np.float64)
    args = ts[:, None] * freqs[None, :]
    temb = np.concatenate([np.sin(args), np.cos(args)], axis=-1)
    t = _silu(temb @ _np(p["t_mlp1"]["w"]) + _np(p["t_mlp1"]["b"]))
    t = t @ _np(p["t_mlp2"]["w"]) + _np(p["t_mlp2"]["b"])         # [B, D]
    st = _silu(t)

    w_in = _np(p["in_proj"]["w"])                                  # [ENC, D]
    pos_full = _np(p["pos_emb"]) + _np(p["in_proj"]["b"])[None, :]  # [N, D]

    fsmod = st @ _np(p["final_mod"]["w"]) + _np(p["final_mod"]["b"])  # [B, 2D]
    fs, fsc = fsmod[:, :D], fsmod[:, D:]

    w_f = _np(p["final"]["w"])
    b_f = _np(p["final"]["b"])

    w1 = _np(p["pair"][0]["w"])     # [2*PROJ, MLP_H]
    b1 = _np(p["pair"][0]["b"])
    w1a, w1b = w1[:PROJ], w1[PROJ:]
    w2p = _np(p["pair"][1]["w"])    # [MLP_H, MLP_H]
    b2p = _np(p["pair"][1]["b"])
    w3 = _np(p["pair_out"]["w"]).reshape(-1)   # [MLP_H]
    b3 = float(np.asarray(p["pair_out"]["b"]).reshape(-1)[0])

    w3blk = np.zeros((128, 8, 4), np.float64)
    for ii in range(4):
        for mc in range(2):
            w3blk[:, ii * 2 + mc, ii] = w3[mc * 128:(mc + 1) * 128]

    eye = np.eye(N, dtype=np.float64)
    mask_half = 0.5 * (1.0 - eye)
    diagm = b3 * (1.0 - eye) - 1e9 * eye

    lw = []
    for l in range(LAYERS):
        lp = p["layers"][l]
        mod = st @ _np(lp["mod"]["w"]) + _np(lp["mod"]["b"])       # [B, 6D]
        s1, sc1, g1, s2, sc2, g2 = np.split(mod, 6, axis=-1)
        lw.append(dict(
            wq=_np(lp["wq"]["w"]), bq=_np(lp["wq"]["b"]),
            wk=_np(lp["wk"]["w"]), bk=_np(lp["wk"]["b"]),
            wv=_np(lp["wv"]["w"]), bv=_np(lp["wv"]["b"]),
            wo=_np(lp["wo"]["w"]), bo=_np(lp["wo"]["b"]),
            w1=_np(lp["mlp1"]["w"]), b1=_np(lp["mlp1"]["b"]),
            w2=_np(lp["mlp2"]["w"]), b2=_np(lp["mlp2"]["b"]),
            s1=s1, sc1=sc1, g1=g1, s2=s2, sc2=sc2, g2=g2,
        ))

    in_maps = []
    sc = np.float32(1.0 / np.sqrt(HD))
    for b in range(B):
        m = {
            "xt": x[b].T,                                    # [ENC, N]
            "w_in": w_in,
            "pos": pos_full,
            "wf": (1.0 + fsc[b])[:, None] * w_f,
            "cf": fs[b] @ w_f + b_f,
            "w1a": w1a, "w1b": w1b, "cu": b1,
            "w2p": w2p, "b2p": b2p, "w3blk": w3blk,
            "mask_half": mask_half, "diagm": diagm,
            "ones_r": np.ones(2 * N, np.float32),
            "b2p_r": np.asarray(b2p, np.float32),
        }
        for l, W in enumerate(lw):
            d1 = (1.0 + W["sc1"][b])[:, None]
            d2 = (1.0 + W["sc2"][b])[:, None]
            m[f"wq{l}"] = d1 * W["wq"] * sc
            m[f"cq{l}"] = (W["s1"][b] @ W["wq"] + W["bq"]) * sc
            m[f"wk{l}"] = d1 * W["wk"]
            m[f"ck{l}"] = W["s1"][b] @ W["wk"] + W["bk"]
            m[f"wv{l}"] = d1 * W["wv"]
            m[f"cv{l}"] = W["s1"][b] @ W["wv"] + W["bv"]
            m[f"wo{l}"] = W["wo"] * W["g1"][b][None, :]
            m[f"co{l}"] = W["bo"] * W["g1"][b]
            m[f"w1{l}"] = d2 * W["w1"]
            m[f"c1{l}"] = W["s2"][b] @ W["w1"] + W["b1"]
            m[f"w2{l}"] = W["w2"] * W["g2"][b][None, :]
            m[f"c2{l}"] = W["b2"] * W["g2"][b]
        m = {k: np.ascontiguousarray(v, dtype=np.float32) for k, v in m.items()}
        in_maps.append(m)
    return in_maps


# ------------------------------------------------------------- device program
def build_program(_debug=False):
    nc = bacc.Bacc("TRN2", target_bir_lowering=False, debug=False)

    def din(name, shape):
        return nc.dram_tensor(name, list(shape), F32, kind="ExternalInput").ap()

    xt_d = nc.dram_tensor("xt", [ENC, N], F32R, kind="ExternalInput").ap()
    win_d = nc.dram_tensor("w_in", [ENC, D], F32R, kind="ExternalInput").ap()
    pos_d = din("pos", (N, D))
    wf_d = nc.dram_tensor("wf", [D, D], F32R, kind="ExternalInput").ap()
    cf_d = din("cf", (D,))
    w1a_d = nc.dram_tensor("w1a", [PROJ, MLP_H], F32R, kind="ExternalInput").ap()
    w1b_d = nc.dram_tensor("w1b", [PROJ, MLP_H], F32R, kind="ExternalInput").ap()
    cu_d = din("cu", (MLP_H,))
    w2p_d = nc.dram_tensor("w2p", [MLP_H, MLP_H], F32R, kind="ExternalInput").ap()
    b2p_d = din("b2p", (MLP_H,))
    w3blk_d = nc.dram_tensor("w3blk", [128, 8, 4], F32R, kind="ExternalInput").ap()
    onesr_d = nc.dram_tensor("ones_r", [2 * N], F32R, kind="ExternalInput").ap()
    b2pr_d = nc.dram_tensor("b2p_r", [MLP_H], F32R, kind="ExternalInput").ap()
    maskh_d = din("mask_half", (N, N))
    diagm_d = din("diagm", (N, N))
    LW = []
    for l in range(LAYERS):
        ent = {}
        for k, shp in [("wq", (D, D)), ("cq", (D,)), ("wk", (D, D)), ("ck", (D,)),
                       ("wv", (D, D)), ("cv", (D,)), ("wo", (D, D)), ("co", (D,)),
                       ("w1", (D, BB_H)), ("c1", (BB_H,)), ("w2", (BB_H, D)), ("c2", (D,))]:
            dt_ = F32R if k in ("wq", "wk", "w1", "wv", "wo", "w2") else F32
            ent[k] = nc.dram_tensor(f"{k}{l}", list(shp), dt_, kind="ExternalInput").ap()
        LW.append(ent)

    logits_d = nc.dram_tensor("logits", [N, N], F32, kind="ExternalOutput").ap()
    x0_d = nc.dram_tensor("x0", [N, N], F32, kind="ExternalOutput").ap()
    DBG = {}
    if _debug:
        for nm, shp in [("dbg_h0", (N, D)), ("dbg_h1", (N, D)), ("dbg_h2", (N, D)),
                        ("dbg_h3", (N, D)), ("dbg_h4", (N, D)),
                        ("dbg_hfT", (D, N)), ("dbg_uT", (D, N)), ("dbg_vT", (D, N)),
                        ("dbg_L", (N, N))]:
            DBG[nm] = nc.dram_tensor(nm, list(shp), F32, kind="ExternalOutput").ap()

    def dump_h(nm):
        if _debug:
            for t, (toff, tsz) in enumerate(TCH):
                nc.sync.dma_start(out=DBG[nm][toff:toff + tsz, :], in_=h_sb[0:tsz, t, :])

    def dump_T(nm, tl):
        if _debug:
            for c in range(2):
                nc.sync.dma_start(out=DBG[nm][c * 128:(c + 1) * 128, :], in_=tl[:, c, :])

    col2 = lambda a: a.rearrange("(c p) -> p c", p=128)     # [2X*128] -> [128, 2X]
    row = lambda a: a.rearrange("(o p) -> o p", o=1)        # [X] -> [1, X]

    with tile.TileContext(nc) as tc:
        with nc.allow_low_precision(reason="f32r intermediates; validated end-to-end"), \
             tc.tile_pool(name="cons", bufs=1) as cons, \
             tc.tile_pool(name="sb", bufs=1) as sb:

            ident = cons.tile([128, 128], F32)
            make_identity(nc, ident)
            ones_row = cons.tile([1, 192], F32)
            nc.vector.memset(ones_row, 1.0)
            ones_row_r = cons.tile([1, 2 * N], F32R)
            nc.sync.dma_start(out=ones_row_r, in_=onesr_d.rearrange("(o p) -> o p", o=1))
            ones_col = cons.tile([128, 1], F32)
            nc.vector.memset(ones_col, 1.0)
            epst = cons.tile([128, 1], F32)
            nc.vector.memset(epst, 1e-6)

            h_sb = cons.tile([128, 2, D], F32)

            def layer_norm(wp, psp, use_newton=False):
                """LN of h_sb -> transposed hlnT [128, 2kc, N]."""
                mv = []
                for t, (toff, tsz) in enumerate(TCH):
                    st_t = wp.tile([128, 6], F32, tag="bnst")
                    nc.vector.bn_stats(out=st_t[0:tsz, :], in_=h_sb[0:tsz, t, :])
                    mv_t = wp.tile([128, 2], F32, tag=f"bnmv{t}")
                    nc.vector.bn_aggr(out=mv_t[0:tsz, :], in_=st_t[0:tsz, :])
                    mv.append(mv_t)
                vpack = wp.tile([128, 2], F32, tag="vpack")
                nc.vector.tensor_copy(out=vpack[:, 0:1], in_=mv[0][:, 1:2])
                nc.vector.tensor_copy(out=vpack[0:64, 1:2], in_=mv[1][0:64, 1:2])
                rstd = wp.tile([128, 2], F32, tag="rstd")
                if not use_newton:
                    lnv = wp.tile([128, 2], F32, tag="lnv")
                    nc.scalar.activation(out=lnv, in_=vpack, func=AF.Ln, bias=epst, scale=1.0)
                    nc.scalar.activation(out=rstd, in_=lnv, func=AF.Exp, bias=0.0, scale=-0.5)
                else:
                    a = wp.tile([128, 2], F32, tag="nta")
                    nc.vector.tensor_scalar(out=a, in0=vpack, scalar1=1e-6, scalar2=None,
                                            op0=ALU.add)
                    xh = wp.tile([128, 2], F32, tag="ntxh")
                    nc.vector.tensor_scalar(out=xh, in0=a, scalar1=0.5, scalar2=None,
                                            op0=ALU.mult)
                    ti = wp.tile([128, 2], I32, tag="nti")
                    nc.vector.tensor_scalar(out=ti, in0=a.bitcast(I32), scalar1=1,
                                            scalar2=None, op0=ALU.logical_shift_right)
                    nc.vector.tensor_scalar(out=ti, in0=ti, scalar1=-1, scalar2=None,
                                            op0=ALU.bitwise_xor)
                    nc.vector.tensor_scalar(out=ti, in0=ti, scalar1=0x5f3759df + 1,
                                            scalar2=None, op0=ALU.add)
                    y = rstd
                    nc.vector.tensor_copy(out=y, in_=ti.bitcast(F32))
                    t1 = wp.tile([128, 2], F32, tag="ntt1")
                    for _ in range(3):
                        nc.vector.tensor_tensor(out=t1, in0=y, in1=y, op=ALU.mult)
                        nc.vector.tensor_tensor(out=t1, in0=t1, in1=xh, op=ALU.mult)
                        nc.vector.tensor_scalar(out=t1, in0=t1, scalar1=-1.0, scalar2=1.5,
                                                op0=ALU.mult, op1=ALU.add)
                        nc.vector.tensor_tensor(out=y, in0=y, in1=t1, op=ALU.mult)
                hln = wp.tile([128, 2, D], F32, tag="hln")
                for t, (toff, tsz) in enumerate(TCH):
                    nc.vector.tensor_scalar(out=hln[0:tsz, t, :], in0=h_sb[0:tsz, t, :],
                                            scalar1=mv[t][0:tsz, 0:1],
                                            scalar2=rstd[0:tsz, t:t + 1],
                                            op0=ALU.subtract, op1=ALU.mult)
                pt = psp.tile([128, 2, 256], F32, tag="ps")
                for kc in range(2):
                    for t, (toff, tsz) in enumerate(TCH):
                        nc.tensor.transpose(pt[0:128, kc, toff:toff + tsz],
                                            hln[0:tsz, t, kc * 128:(kc + 1) * 128],
                                            ident[0:tsz, 0:tsz])
                hlnT = wp.tile([128, 2, 256], F32R, tag="hlnT")
                nc.vector.tensor_copy(out=hlnT[:, :, 0:N], in_=pt[:, :, 0:N])
                return hlnT

            # ================= in_proj =================
            with tc.tile_pool(name="inp", bufs=1) as inp, \
                 tc.tile_pool(name="psA", bufs=2, space="PSUM") as psA:
                xt_sb = inp.tile([128, NEC, N], F32R)
                win_sb = inp.tile([128, NEC, D], F32R)
                pos_sb = inp.tile([128, 2, D], F32)
                for c, (off, sz) in enumerate(ECH):
                    nc.sync.dma_start(out=xt_sb[0:sz, c, :], in_=xt_d[off:off + sz, :])
                    nc.sync.dma_start(out=win_sb[0:sz, c, :], in_=win_d[off:off + sz, :])
                for t, (toff, tsz) in enumerate(TCH):
                    nc.sync.dma_start(out=pos_sb[0:tsz, t, :], in_=pos_d[toff:toff + tsz, :])
                for t, (toff, tsz) in enumerate(TCH):
                    ph = psA.tile([128, D], F32, tag="ph")
                    for c, (off, sz) in enumerate(ECH):
                        if tsz == 128:
                            nc.tensor.matmul(ph[0:tsz, :], xt_sb[0:sz, c, toff:toff + tsz],
                                             win_sb[0:sz, c, :],
                                             start=(c == 0), stop=(c == NEC - 1))
                        else:
                            nc.tensor.matmul(ph[0:tsz, :],
                                             xt_sb.bitcast(F32)[0:sz, c, toff:toff + tsz],
                                             win_sb.bitcast(F32)[0:sz, c, :],
                                             start=(c == 0), stop=(c == NEC - 1))
                    nc.vector.tensor_tensor(out=h_sb[0:tsz, t, :], in0=ph[0:tsz, :],
                                            in1=pos_sb[0:tsz, t, :], op=ALU.add)
                dump_h("dbg_h0")

            # ================= backbone =================
            with tc.tile_pool(name="wpool", bufs=2) as wpool, \
                 tc.tile_pool(name="tpool", bufs=2) as tpool, \
                 tc.tile_pool(name="psB", bufs=6, space="PSUM") as psB, \
                 tc.tile_pool(name="psS", bufs=2, space="PSUM") as psSm:
                for l in range(LAYERS):
                    wd = LW[l]
                    wq = wpool.tile([128, 2, D], F32R, tag="wq")
                    wk = wpool.tile([128, 2, D], F32R, tag="wk")
                    wv = wpool.tile([128, 2, D], F32R, tag="wv")
                    wo = wpool.tile([128, 2, D], F32R, tag="wo")
                    w1 = wpool.tile([128, 2, BB_H], F32R, tag="w1")
                    w2 = wpool.tile([128, 8, D], F32R, tag="w2")
                    for w_sb, w_dr in [(wq, wd["wq"]), (wk, wd["wk"]), (wv, wd["wv"]),
                                       (wo, wd["wo"]), (w1, wd["w1"]), (w2, wd["w2"])]:
                        nc.sync.dma_start(out=w_sb, in_=w_dr.rearrange("(c p) m -> p c m", p=128))
                    cq = wpool.tile([128, 2], F32, tag="cq")
                    nc.sync.dma_start(out=cq, in_=col2(wd["cq"]))
                    ck = wpool.tile([128, 2], F32, tag="ck")
                    nc.sync.dma_start(out=ck, in_=col2(wd["ck"]))
                    c1 = wpool.tile([128, 8], F32, tag="c1")
                    nc.sync.dma_start(out=c1, in_=col2(wd["c1"]))
                    cv = wpool.tile([1, D], F32, tag="cv")
                    nc.sync.dma_start(out=cv, in_=row(wd["cv"]))
                    co = wpool.tile([1, D], F32, tag="co")
                    nc.sync.dma_start(out=co, in_=row(wd["co"]))
                    c2 = wpool.tile([1, D], F32, tag="c2")
                    nc.sync.dma_start(out=c2, in_=row(wd["c2"]))

                    # --- attention ---
                    hlnT = layer_norm(tpool, psB)
                    qT = tpool.tile([128, 2, N], F32, tag="qT")
                    kT = tpool.tile([128, 2, N], F32, tag="kT")
                    for w_sb, c_sb, out_sb in [(wq, cq, qT), (wk, ck, kT)]:
                        pq = psB.tile([128, 2, 256], F32, tag="ps")
                        for mc in range(2):
                            for kc in range(2):
                                nc.tensor.matmul(pq[:, mc, :],
                                                 w_sb[:, kc, mc * 128:(mc + 1) * 128],
                                                 hlnT[:, kc, :], start=(kc == 0), stop=(kc == 1))
                        for mc in range(2):
                            nc.vector.tensor_scalar(out=out_sb[:, mc, :], in0=pq[:, mc, 0:N],
                                                    scalar1=c_sb[:, mc:mc + 1], scalar2=None,
                                                    op0=ALU.add)
                    pv = psB.tile([128, 2, 256], F32, tag="ps")
                    for t, (toff, tsz) in enumerate(TCH):
                        for kc in range(2):
                            if tsz == 128:
                                nc.tensor.matmul(pv[0:tsz, t, :],
                                                 hlnT[:, kc, toff:toff + tsz],
                                                 wv[:, kc, :], start=(kc == 0), stop=False)
                            else:
                                nc.tensor.matmul(pv[0:tsz, t, :],
                                                 hlnT.bitcast(F32)[:, kc, toff:toff + tsz],
                                                 wv.bitcast(F32)[:, kc, :],
                                                 start=(kc == 0), stop=False)
                        nc.tensor.matmul(pv[0:tsz, t, :], ones_row[0:1, 0:tsz], cv,
                                         start=False, stop=True)
                    v_sb = tpool.tile([128, 2, D], F32, tag="v_sb")
                    nc.vector.tensor_copy(out=v_sb, in_=pv)

                    ET = tpool.tile([128, 2, HEADS, N], F32, tag="ET")
                    for hh in range(HEADS):
                        hp, mc = 64 * (hh % 2), hh // 2
                        pss = psB.tile([128, 2, 256], F32, tag="ps")
                        for t2, (t2off, t2sz) in enumerate(TCH):
                            nc.tensor.matmul(pss[0:t2sz, t2, 0:N],
                                             kT[hp:hp + 64, mc, t2off:t2off + t2sz],
                                             qT[hp:hp + 64, mc, :], start=True, stop=True)
                        nc.scalar.activation(out=ET[:, :, hh, :], in_=pss[:, :, 0:N],
                                             func=AF.Exp, bias=0.0, scale=1.0)
                    for hpair in range(2):
                        prs = psSm.tile([1, 2, 256], F32, tag="pr")
                        for hi in range(2):
                            hh = hpair * 2 + hi
                            for t2, (t2off, t2sz) in enumerate(TCH):
                                nc.tensor.matmul(prs[0:1, hi, 0:N], ones_col[0:t2sz, 0:1],
                                                 ET[0:t2sz, t2, hh, :],
                                                 start=(t2 == 0), stop=(t2 == 1))
                        rsr = tpool.tile([1, 2, N], F32, tag="rsr")
                        nc.vector.reciprocal(out=rsr, in_=prs[0:1, :, 0:N])
                        pbc = psB.tile([128, 2, 256], F32, tag="ps")
                        for hi in range(2):
                            nc.tensor.matmul(pbc[:, hi, 0:N], ones_row[0:1, 0:128],
                                             rsr[0:1, hi, :], start=True, stop=True)
                        for t2 in range(2):
                            nc.vector.tensor_tensor(
                                out=ET[:, t2, hpair * 2:hpair * 2 + 2, :],
                                in0=ET[:, t2, hpair * 2:hpair * 2 + 2, :],
                                in1=pbc[:, :, 0:N], op=ALU.mult)
                    po = psB.tile([128, 2, 256], F32, tag="ps")
                    for hh in range(HEADS):
                        hp, s = 64 * (hh % 2), hh // 2
                        for t2, (t2off, t2sz) in enumerate(TCH):
                            nc.tensor.matmul(po[hp:hp + 64, s, 0:N],
                                             v_sb[0:t2sz, t2, hh * 64:hh * 64 + 64],
                                             ET[0:t2sz, t2, hh, :],
                                             start=(t2 == 0), stop=(t2 == 1),
                                             tile_position=(0, hp))
                    oT = tpool.tile([128, 2, N], F32R, tag="oT")
                    nc.vector.tensor_copy(out=oT, in_=po[:, :, 0:N])
                    pa = psB.tile([128, 2, 256], F32, tag="ps")
                    for t, (toff, tsz) in enumerate(TCH):
                        for kc in range(2):
                            if tsz == 128:
                                nc.tensor.matmul(pa[0:tsz, t, :], oT[:, kc, toff:toff + tsz],
                                                 wo[:, kc, :], start=(kc == 0), stop=False)
                            else:
                                nc.tensor.matmul(pa[0:tsz, t, :],
                                                 oT.bitcast(F32)[:, kc, toff:toff + tsz],
                                                 wo.bitcast(F32)[:, kc, :],
                                                 start=(kc == 0), stop=False)
                        nc.tensor.matmul(pa[0:tsz, t, :], ones_row[0:1, 0:tsz], co,
                                         start=False, stop=True)
                    for t, (toff, tsz) in enumerate(TCH):
                        nc.vector.tensor_tensor(out=h_sb[0:tsz, t, :], in0=h_sb[0:tsz, t, :],
                                                in1=pa[0:tsz, t, :], op=ALU.add)

                    # --- mlp ---
                    hln2T = layer_norm(tpool, psB)
                    gm1T = tpool.tile([128, 8, N], F32R, tag="gm1T")
                    for mc in range(8):
                        pm = psB.tile([128, 256], F32, tag="ps")
                        for kc in range(2):
                            nc.tensor.matmul(pm[:, :],
                                             w1[:, kc, mc * 128:(mc + 1) * 128],
                                             hln2T[:, kc, :],
                                             start=(kc == 0), stop=(kc == 1))
                        nc.scalar.activation(out=gm1T[:, mc, :], in_=pm[:, 0:N],
                                             func=AF.Gelu_apprx_tanh,
                                             bias=c1[:, mc:mc + 1], scale=1.0)
                    pa2 = psB.tile([128, 2, 256], F32, tag="ps")
                    for t, (toff, tsz) in enumerate(TCH):
                        for kc in range(8):
                            if tsz == 128:
                                nc.tensor.matmul(pa2[0:tsz, t, :], gm1T[:, kc, toff:toff + tsz],
                                                 w2[:, kc, :], start=(kc == 0), stop=False)
                            else:
                                nc.tensor.matmul(pa2[0:tsz, t, :],
                                                 gm1T.bitcast(F32)[:, kc, toff:toff + tsz],
                                                 w2.bitcast(F32)[:, kc, :],
                                                 start=(kc == 0), stop=False)
                        nc.tensor.matmul(pa2[0:tsz, t, :], ones_row[0:1, 0:tsz], c2,
                                         start=False, stop=True)
                    for t, (toff, tsz) in enumerate(TCH):
                        nc.vector.tensor_tensor(out=h_sb[0:tsz, t, :], in0=h_sb[0:tsz, t, :],
                                                in1=pa2[0:tsz, t, :], op=ALU.add)
                    dump_h(f"dbg_h{l + 1}")

                # ============ final LN + projections (inside backbone pools) ============
                wf_sb = sb.tile([128, 2, D], F32R)
                nc.sync.dma_start(out=wf_sb, in_=wf_d.rearrange("(c p) m -> p c m", p=128))
                cf_sb = sb.tile([128, 2], F32)
                nc.sync.dma_start(out=cf_sb, in_=col2(cf_d))
                w1a_sb = sb.tile([128, 2, MLP_H], F32R)
                nc.sync.dma_start(out=w1a_sb, in_=w1a_d.rearrange("(c p) m -> p c m", p=128))
                w1b_sb = sb.tile([128, 2, MLP_H], F32R)
                nc.sync.dma_start(out=w1b_sb, in_=w1b_d.rearrange("(c p) m -> p c m", p=128))
                cu_sb = sb.tile([128, 2], F32)
                nc.sync.dma_start(out=cu_sb, in_=col2(cu_d))

                hlnfT = layer_norm(tpool, psB, use_newton=True)
                uT = sb.tile([128, 2, N], F32)
                vT = sb.tile([128, 2, N], F32)
                hfT = sb.tile([128, 2, 256], F32R)
                pf = psB.tile([128, 2, 256], F32, tag="ps")
                for mc in range(2):
                    for kc in range(2):
                        nc.tensor.matmul(pf[:, mc, :], wf_sb[:, kc, mc * 128:(mc + 1) * 128],
                                         hlnfT[:, kc, :], start=(kc == 0), stop=(kc == 1))
                for mc in range(2):
                    nc.vector.tensor_scalar(out=hfT[:, mc, 0:N], in0=pf[:, mc, 0:N],
                                            scalar1=cf_sb[:, mc:mc + 1], scalar2=None,
                                            op0=ALU.add)
                pu = psB.tile([128, 2, 256], F32, tag="ps")
                pvv = psB.tile([128, 2, 256], F32, tag="ps")
                for mc in range(2):
                    for kc in range(2):
                        nc.tensor.matmul(pu[:, mc, :], w1a_sb[:, kc, mc * 128:(mc + 1) * 128],
                                         hfT[:, kc, :], start=(kc == 0), stop=(kc == 1))
                        nc.tensor.matmul(pvv[:, mc, :], w1b_sb[:, kc, mc * 128:(mc + 1) * 128],
                                         hfT[:, kc, :], start=(kc == 0), stop=(kc == 1))
                for mc in range(2):
                    nc.vector.tensor_scalar(out=uT[:, mc, :], in0=pu[:, mc, 0:N],
                                            scalar1=cu_sb[:, mc:mc + 1], scalar2=None,
                                            op0=ALU.add)
                nc.vector.tensor_copy(out=vT, in_=pvv[:, :, 0:N])
                dump_T("dbg_hfT", hfT)
                dump_T("dbg_uT", uT)
                dump_T("dbg_vT", vT)

            # ================= pairwise MLP =================
            w2p_sb = sb.tile([128, 2, MLP_H], F32R)
            nc.sync.dma_start(out=w2p_sb, in_=w2p_d.rearrange("(c p) m -> p c m", p=128))
            b2p_sb = sb.tile([128, 2], F32)
            nc.sync.dma_start(out=b2p_sb, in_=col2(b2p_d))
            b2pr_sb = sb.tile([1, MLP_H], F32R)
            nc.sync.dma_start(out=b2pr_sb, in_=b2pr_d.rearrange("(o p) -> o p", o=1))
            w3b_sb = sb.tile([128, 8, 4], F32R)
            nc.sync.dma_start(out=w3b_sb, in_=w3blk_d)
            maskh_sb = sb.tile([128, 2, N], F32)
            diagm_sb = sb.tile([128, 2, N], F32)
            for t, (toff, tsz) in enumerate(TCH):
                nc.sync.dma_start(out=maskh_sb[0:tsz, t, :], in_=maskh_d[toff:toff + tsz, :])
                nc.sync.dma_start(out=diagm_sb[0:tsz, t, :], in_=diagm_d[toff:toff + tsz, :])

            L_sb = sb.tile([128, 2, N], F32)
            with tc.tile_pool(name="zp", bufs=2) as zp, \
                 tc.tile_pool(name="gzp", bufs=2) as gzp, \
                 tc.tile_pool(name="g2p", bufs=3) as g2p, \
                 tc.tile_pool(name="sc4", bufs=3) as sc4p, \
                 tc.tile_pool(name="pps", bufs=3, space="PSUM") as pps, \
                 tc.tile_pool(name="plt", bufs=2, space="PSUM") as plt:
                for ib in range(N // IBLK):
                    z = zp.tile([128, 2, IBLK * N], F32)
                    gz = gzp.tile([128, 2, IBLK * N], F32R)
                    for ii in range(IBLK):
                        i = ib * IBLK + ii
                        for kc in range(2):
                            nc.vector.tensor_scalar(out=z[:, kc, ii * N:(ii + 1) * N],
                                                    in0=vT[:, kc, :],
                                                    scalar1=uT[:, kc, i:i + 1],
                                                    scalar2=None, op0=ALU.add)
                    for ih in range(4):
                        sl = slice(ih * (IBLK // 4) * N, (ih + 1) * (IBLK // 4) * N)
                        nc.scalar.activation(out=gz[:, :, sl], in_=z[:, :, sl],
                                             func=AF.Gelu_apprx_tanh, bias=0.0, scale=1.0)
                    for g4 in range(IBLK // 4):
                        g2sb = g2p.tile([128, 2, 4, 256], F32R)
                        for pp in range(2):
                            # [128, mc, 512-pad]: one accumulation group per bank,
                            # N=384 covers two consecutive i's in one matmul
                            ps2 = pps.tile([128, 2, 512], F32)
                            iioff = (g4 * 4 + pp * 2) * N
                            for mc in range(2):
                                for kc in range(2):
                                    nc.tensor.matmul(
                                        ps2[:, mc, 0:2 * N],
                                        w2p_sb[:, kc, mc * 128:(mc + 1) * 128],
                                        gz[:, kc, iioff:iioff + 2 * N],
                                        start=(kc == 0), stop=False)
                                nc.tensor.matmul(
                                    ps2[:, mc, 0:2 * N],
                                    b2pr_sb[0:1, mc * 128:(mc + 1) * 128],
                                    ones_row_r[0:1, 0:2 * N],
                                    start=False, stop=True)
                            nc.scalar.activation(
                                out=g2sb[:, :, pp * 2:pp * 2 + 2, 0:N],
                                in_=ps2[:, :, 0:2 * N].rearrange(
                                    "p a (b n) -> p a b n", n=N),
                                func=AF.Gelu_apprx_tanh, bias=0.0, scale=1.0)
                        lt = plt.tile([4, 256], F32)
                        for kc8 in range(8):
                            ii, mc = kc8 // 2, kc8 % 2
                            nc.tensor.matmul(lt[0:4, :], w3b_sb[:, kc8, :],
                                             g2sb[:, mc, ii, :],
                                             start=(kc8 == 0), stop=(kc8 == 7))
                        i0 = ib * IBLK + g4 * 4
                        t, r = i0 // 128, i0 % 128
                        sc4 = sc4p.tile([4, N], F32)
                        nc.vector.tensor_copy(out=sc4, in_=lt[0:4, 0:N])
                        nc.sync.dma_start(out=L_sb[r:r + 4, t, :], in_=sc4)

            if _debug:
                for t, (toff, tsz) in enumerate(TCH):
                    nc.sync.dma_start(out=DBG["dbg_L"][toff:toff + tsz, :], in_=L_sb[0:tsz, t, :])
            # ================= symmetrize + logits + P0 + Sinkhorn =================
            with tc.tile_pool(name="psC", bufs=2, space="PSUM") as psC, \
                 tc.tile_pool(name="psV", bufs=2, space="PSUM") as psV:
                psL = psC.tile([128, 2, 256], F32, tag="ps")
                for t_out, (joff, jsz) in enumerate(TCH):
                    for t_in, (ioff, isz) in enumerate(TCH):
                        nc.tensor.transpose(psL[0:jsz, t_out, ioff:ioff + isz],
                                            L_sb[0:isz, t_in, joff:joff + jsz],
                                            ident[0:isz, 0:isz])
                Lsym = sb.tile([128, 2, N], F32)
                nc.vector.tensor_tensor(out=Lsym, in0=L_sb, in1=psL[:, :, 0:N], op=ALU.add)
                Lmh = sb.tile([128, 2, N], F32)
                nc.vector.tensor_tensor(out=Lmh, in0=Lsym, in1=maskh_sb, op=ALU.mult)
                lo = sb.tile([128, 2, N], F32)
                nc.vector.tensor_tensor(out=lo, in0=Lmh, in1=diagm_sb, op=ALU.add)
                for t, (toff, tsz) in enumerate(TCH):
                    nc.sync.dma_start(out=logits_d[toff:toff + tsz, :], in_=lo[0:tsz, t, :])

                mrow = sb.tile([128, 2], F32)
                for t, (toff, tsz) in enumerate(TCH):
                    nc.vector.reduce_max(out=mrow[0:tsz, t:t + 1], in_=Lmh[0:tsz, t, :],
                                         axis=AX.X)
                pmr = psV.tile([1, 2, 128], F32, tag="pv")
                for t, (toff, tsz) in enumerate(TCH):
                    nc.tensor.transpose(pmr[0:1, t, 0:tsz], mrow[0:tsz, t:t + 1],
                                        ident[0:tsz, 0:tsz])
                mjoin = sb.tile([1, N], F32)
                nc.vector.tensor_copy(out=mjoin[0:1, 0:128], in_=pmr[0:1, 0, :])
                nc.vector.tensor_copy(out=mjoin[0:1, 128:192], in_=pmr[0:1, 1, 0:64])
                gmax = sb.tile([1, 1], F32)
                nc.vector.reduce_max(out=gmax, in_=mjoin, axis=AX.X)
                pgb = psV.tile([128, 1], F32, tag="pg")
                nc.tensor.matmul(pgb, ones_row[0:1, 0:128], gmax, start=True, stop=True)
                negb = sb.tile([128, 1], F32)
                nc.vector.tensor_scalar(out=negb, in0=pgb, scalar1=-20.0, scalar2=None,
                                        op0=ALU.mult)

                P0 = sb.tile([128, 2, N], F32)
                nc.scalar.activation(out=P0, in_=lo, func=AF.Exp, bias=negb, scale=20.0)

                ccol0 = sb.tile([128, 2], F32)
                nc.vector.memset(ccol0, 1.0)
                cur = ccol0
                rcol = ccol = None
                for it in range(SINK_ITERS):
                    for half in range(2):
                        pd = psV.tile([128, 2], F32, tag="pd")
                        for mc, (moff, msz) in enumerate(TCH):
                            for kc, (koff, ksz) in enumerate(TCH):
                                nc.tensor.matmul(pd[0:msz, mc:mc + 1],
                                                 P0[0:ksz, kc, moff:moff + msz],
                                                 cur[0:ksz, kc:kc + 1],
                                                 start=(kc == 0), stop=(kc == 1))
                        nxt = sb.tile([128, 2], F32, tag=f"rc{half}{it % 2}")
                        nc.vector.reciprocal(out=nxt, in_=pd)
                        if half == 0:
                            rcol = nxt
                        else:
                            ccol = nxt
                        cur = nxt

                prr = psV.tile([1, 2, 128], F32, tag="pv")
                pcc = psV.tile([1, 2, 128], F32, tag="pv")
                for t, (toff, tsz) in enumerate(TCH):
                    nc.tensor.transpose(prr[0:1, t, 0:tsz], rcol[0:tsz, t:t + 1],
                                        ident[0:tsz, 0:tsz])
                    nc.tensor.transpose(pcc[0:1, t, 0:tsz], ccol[0:tsz, t:t + 1],
                                        ident[0:tsz, 0:tsz])
                rrow = sb.tile([1, N], F32)
                crow = sb.tile([1, N], F32)
                for t, (toff, tsz) in enumerate(TCH):
                    nc.vector.tensor_scalar(out=rrow[0:1, toff:toff + tsz],
                                            in0=prr[0:1, t, 0:tsz],
                                            scalar1=0.5, scalar2=None, op0=ALU.mult)
                    nc.vector.tensor_copy(out=crow[0:1, toff:toff + tsz],
                                          in_=pcc[0:1, t, 0:tsz])
                psS = psC.tile([128, 2, 256], F32, tag="ps")
                for mc, (moff, msz) in enumerate(TCH):
                    nc.tensor.matmul(psS[0:msz, mc, 0:N], rrow[0:1, moff:moff + msz], crow,
                                     start=True, stop=False)
                    nc.tensor.matmul(psS[0:msz, mc, 0:N], crow[0:1, moff:moff + msz], rrow,
                                     start=False, stop=True)
                x0sb = sb.tile([128, 2, N], F32)
                nc.vector.tensor_tensor(out=x0sb, in0=P0, in1=psS[:, :, 0:N], op=ALU.mult)
                for t, (toff, tsz) in enumerate(TCH):
                    nc.sync.dma_start(out=x0_d[toff:toff + tsz, :], in_=x0sb[0:tsz, t, :])

    nc.compile()
    return nc


_PROGRAM = None


def kernel(x, timesteps, params, _trace=False):
    global _PROGRAM
    in_maps = _host_prep(x, timesteps, params)
    if _PROGRAM is None:
        _PROGRAM = build_program()
    res = run_bass_kernel_spmd(_PROGRAM, in_maps, list(range(B)), trace=_trace)
    _PROFILE["last"] = res
    logits = np.stack([res.results[b]["logits"] for b in range(B)])
    x0 = np.stack([res.results[b]["x0"] for b in range(B)])
    return logits, x0


# revision 34
# speedup vs baseline: 1.2103x; 1.0036x over previous
"""Trainium2 Bass kernel for nn_DiscreteDiT (DiT backbone + pairwise MLP + Sinkhorn).

Self-contained: hardcodes all shapes. Shards batch B=8 across 8 NeuronCores
(one batch element per core, params replicated / host-prefolded per batch).
"""
import sys

for _p in ("/opt/trn_rl_repo", "/root/.axon_site/_ro/trn_rl_repo"):
    if _p not in sys.path:
        sys.path.insert(0, _p)

import numpy as np

import concourse.bacc as bacc
import concourse.mybir as mybir
import concourse.tile as tile
from concourse.bass_utils import run_bass_kernel_spmd
from concourse.masks import make_identity

F32 = mybir.dt.float32
F32R = mybir.dt.float32r
I32 = mybir.dt.int32
AF = mybir.ActivationFunctionType
ALU = mybir.AluOpType
AX = mybir.AxisListType

N = 192          # nodes / tokens
ENC = 1728       # encoding dim
D = 256          # d_model
LAYERS = 4
HEADS = 4
HD = 64
PROJ = 256
MLP_H = 256      # pair MLP hidden
BB_H = 1024      # backbone MLP hidden
SINK_ITERS = 20
B = 8

TCH = [(0, 128), (128, 64)]            # token chunks (start, size)
ECH = [(i * 128, 128) for i in range(13)] + [(1664, 64)]   # ENC chunks
NEC = len(ECH)
IBLK = 16                               # pair-stage i block

_PROFILE = {}


# ----------------------------------------------------------------- host prep
def _silu(x):
    return x / (1.0 + np.exp(-x))


def _np(a):
    return np.asarray(a, dtype=np.float64)


def _host_prep(x, timesteps, params):
    """Returns list of 8 per-core input dicts."""
    x = np.asarray(x, np.float64)
    ts = np.asarray(timesteps).astype(np.float64)
    p = params

    half = D // 2
    # match reference: freqs computed in fp32, args/sin/cos then in fp64 for accuracy
    freqs = np.exp(-np.log(np.float32(10000.0)) * np.arange(half, dtype=np.float32)
                   / np.float32(half)).astype(np.float32).astype(np.float64)
    args = ts[:, None] * freqs[None, :]
    temb = np.concatenate([np.sin(args), np.cos(args)], axis=-1)
    t = _silu(temb @ _np(p["t_mlp1"]["w"]) + _np(p["t_mlp1"]["b"]))
    t = t @ _np(p["t_mlp2"]["w"]) + _np(p["t_mlp2"]["b"])         # [B, D]
    st = _silu(t)

    w_in = _np(p["in_proj"]["w"])                                  # [ENC, D]
    pos_full = _np(p["pos_emb"]) + _np(p["in_proj"]["b"])[None, :]  # [N, D]

    fsmod = st @ _np(p["final_mod"]["w"]) + _np(p["final_mod"]["b"])  # [B, 2D]
    fs, fsc = fsmod[:, :D], fsmod[:, D:]

    w_f = _np(p["final"]["w"])
    b_f = _np(p["final"]["b"])

    w1 = _np(p["pair"][0]["w"])     # [2*PROJ, MLP_H]
    b1 = _np(p["pair"][0]["b"])
    w1a, w1b = w1[:PROJ], w1[PROJ:]
    w2p = _np(p["pair"][1]["w"])    # [MLP_H, MLP_H]
    b2p = _np(p["pair"][1]["b"])
    w3 = _np(p["pair_out"]["w"]).reshape(-1)   # [MLP_H]
    b3 = float(np.asarray(p["pair_out"]["b"]).reshape(-1)[0])

    w3blk = np.zeros((128, 8, 4), np.float64)
    for ii in range(4):
        for mc in range(2):
            w3blk[:, ii * 2 + mc, ii] = w3[mc * 128:(mc + 1) * 128]

    eye = np.eye(N, dtype=np.float64)
    mask_half = 0.5 * (1.0 - eye)
    diagm = b3 * (1.0 - eye) - 1e9 * eye

    lw = []
    for l in range(LAYERS):
        lp = p["layers"][l]
        mod = st @ _np(lp["mod"]["w"]) + _np(lp["mod"]["b"])       # [B, 6D]
        s1, sc1, g1, s2, sc2, g2 = np.split(mod, 6, axis=-1)
        lw.append(dict(
            wq=_np(lp["wq"]["w"]), bq=_np(lp["wq"]["b"]),
            wk=_np(lp["wk"]["w"]), bk=_np(lp["wk"]["b"]),
            wv=_np(lp["wv"]["w"]), bv=_np(lp["wv"]["b"]),
            wo=_np(lp["wo"]["w"]), bo=_np(lp["wo"]["b"]),
            w1=_np(lp["mlp1"]["w"]), b1=_np(lp["mlp1"]["b"]),
            w2=_np(lp["mlp2"]["w"]), b2=_np(lp["mlp2"]["b"]),
            s1=s1, sc1=sc1, g1=g1, s2=s2, sc2=sc2, g2=g2,
        ))

    in_maps = []
    sc = np.float32(1.0 / np.sqrt(HD))
    for b in range(B):
        m = {
            "xt": x[b].T,                                    # [ENC, N]
            "w_in": w_in,
            "pos": pos_full,
            "wf": (1.0 + fsc[b])[:, None] * w_f,
            "cf": fs[b] @ w_f + b_f,
            "w1a": w1a, "w1b": w1b, "cu": b1,
            "w2p": w2p, "b2p": b2p, "w3blk": w3blk,
            "mask_half": mask_half, "diagm": diagm,
            "ones_r": np.ones(2 * N, np.float32),
            "b2p_r": np.asarray(b2p, np.float32),
        }
        for l, W in enumerate(lw):
            d1 = (1.0 + W["sc1"][b])[:, None]
            d2 = (1.0 + W["sc2"][b])[:, None]
            m[f"wq{l}"] = d1 * W["wq"] * sc
            m[f"cq{l}"] = (W["s1"][b] @ W["wq"] + W["bq"]) * sc
            m[f"wk{l}"] = d1 * W["wk"]
            m[f"ck{l}"] = W["s1"][b] @ W["wk"] + W["bk"]
            m[f"wv{l}"] = d1 * W["wv"]
            m[f"cv{l}"] = W["s1"][b] @ W["wv"] + W["bv"]
            m[f"wo{l}"] = W["wo"] * W["g1"][b][None, :]
            m[f"co{l}"] = W["bo"] * W["g1"][b]
            m[f"w1{l}"] = d2 * W["w1"]
            m[f"c1{l}"] = W["s2"][b] @ W["w1"] + W["b1"]
            m[f"w2{l}"] = W["w2"] * W["g2"][b][None, :]
            m[f"c2{l}"] = W["b2"] * W["g2"][b]
        m = {k: np.ascontiguousarray(v, dtype=np.float32) for k, v in m.items()}
        in_maps.append(m)
    return in_maps


# ------------------------------------------------------------- device program
def build_program(_debug=False):
    nc = bacc.Bacc("TRN2", target_bir_lowering=False, debug=False)

    def din(name, shape):
        return nc.dram_tensor(name, list(shape), F32, kind="ExternalInput").ap()

    xt_d = nc.dram_tensor("xt", [ENC, N], F32R, kind="ExternalInput").ap()
    win_d = nc.dram_tensor("w_in", [ENC, D], F32R, kind="ExternalInput").ap()
    pos_d = din("pos", (N, D))
    wf_d = nc.dram_tensor("wf", [D, D], F32R, kind="ExternalInput").ap()
    cf_d = din("cf", (D,))
    w1a_d = nc.dram_tensor("w1a", [PROJ, MLP_H], F32R, kind="ExternalInput").ap()
    w1b_d = nc.dram_tensor("w1b", [PROJ, MLP_H], F32R, kind="ExternalInput").ap()
    cu_d = din("cu", (MLP_H,))
    w2p_d = nc.dram_tensor("w2p", [MLP_H, MLP_H], F32R, kind="ExternalInput").ap()
    b2p_d = din("b2p", (MLP_H,))
    w3blk_d = nc.dram_tensor("w3blk", [128, 8, 4], F32R, kind="ExternalInput").ap()
    onesr_d = nc.dram_tensor("ones_r", [2 * N], F32R, kind="ExternalInput").ap()
    b2pr_d = nc.dram_tensor("b2p_r", [MLP_H], F32R, kind="ExternalInput").ap()
    maskh_d = din("mask_half", (N, N))
    diagm_d = din("diagm", (N, N))
    LW = []
    for l in range(LAYERS):
        ent = {}
        for k, shp in [("wq", (D, D)), ("cq", (D,)), ("wk", (D, D)), ("ck", (D,)),
                       ("wv", (D, D)), ("cv", (D,)), ("wo", (D, D)), ("co", (D,)),
                       ("w1", (D, BB_H)), ("c1", (BB_H,)), ("w2", (BB_H, D)), ("c2", (D,))]:
            dt_ = F32R if k in ("wq", "wk", "w1", "wv", "wo", "w2") else F32
            ent[k] = nc.dram_tensor(f"{k}{l}", list(shp), dt_, kind="ExternalInput").ap()
        LW.append(ent)

    logits_d = nc.dram_tensor("logits", [N, N], F32, kind="ExternalOutput").ap()
    x0_d = nc.dram_tensor("x0", [N, N], F32, kind="ExternalOutput").ap()
    DBG = {}
    if _debug:
        for nm, shp in [("dbg_h0", (N, D)), ("dbg_h1", (N, D)), ("dbg_h2", (N, D)),
                        ("dbg_h3", (N, D)), ("dbg_h4", (N, D)),
                        ("dbg_hfT", (D, N)), ("dbg_uT", (D, N)), ("dbg_vT", (D, N)),
                        ("dbg_L", (N, N))]:
            DBG[nm] = nc.dram_tensor(nm, list(shp), F32, kind="ExternalOutput").ap()

    def dump_h(nm):
        if _debug:
            for t, (toff, tsz) in enumerate(TCH):
                nc.sync.dma_start(out=DBG[nm][toff:toff + tsz, :], in_=h_sb[0:tsz, t, :])

    def dump_T(nm, tl):
        if _debug:
            for c in range(2):
                nc.sync.dma_start(out=DBG[nm][c * 128:(c + 1) * 128, :], in_=tl[:, c, :])

    col2 = lambda a: a.rearrange("(c p) -> p c", p=128)     # [2X*128] -> [128, 2X]
    row = lambda a: a.rearrange("(o p) -> o p", o=1)        # [X] -> [1, X]

    with tile.TileContext(nc) as tc:
        with nc.allow_low_precision(reason="f32r intermediates; validated end-to-end"), \
             tc.tile_pool(name="cons", bufs=1) as cons, \
             tc.tile_pool(name="sb", bufs=1) as sb:

            ident = cons.tile([128, 128], F32)
            make_identity(nc, ident)
            ones_row = cons.tile([1, 192], F32)
            nc.vector.memset(ones_row, 1.0)
            ones_row_r = cons.tile([1, 2 * N], F32R)
            nc.sync.dma_start(out=ones_row_r, in_=onesr_d.rearrange("(o p) -> o p", o=1))
            ones_col = cons.tile([128, 1], F32)
            nc.vector.memset(ones_col, 1.0)
            epst = cons.tile([128, 1], F32)
            nc.vector.memset(epst, 1e-6)

            h_sb = cons.tile([128, 2, D], F32)

            def layer_norm(wp, psp, use_newton=False):
                """LN of h_sb -> transposed hlnT [128, 2kc, N]."""
                mv = []
                for t, (toff, tsz) in enumerate(TCH):
                    st_t = wp.tile([128, 6], F32, tag="bnst")
                    nc.vector.bn_stats(out=st_t[0:tsz, :], in_=h_sb[0:tsz, t, :])
                    mv_t = wp.tile([128, 2], F32, tag=f"bnmv{t}")
                    nc.vector.bn_aggr(out=mv_t[0:tsz, :], in_=st_t[0:tsz, :])
                    mv.append(mv_t)
                vpack = wp.tile([128, 2], F32, tag="vpack")
                nc.vector.tensor_copy(out=vpack[:, 0:1], in_=mv[0][:, 1:2])
                nc.vector.tensor_copy(out=vpack[0:64, 1:2], in_=mv[1][0:64, 1:2])
                rstd = wp.tile([128, 2], F32, tag="rstd")
                if not use_newton:
                    lnv = wp.tile([128, 2], F32, tag="lnv")
                    nc.scalar.activation(out=lnv, in_=vpack, func=AF.Ln, bias=epst, scale=1.0)
                    nc.scalar.activation(out=rstd, in_=lnv, func=AF.Exp, bias=0.0, scale=-0.5)
                else:
                    a = wp.tile([128, 2], F32, tag="nta")
                    nc.vector.tensor_scalar(out=a, in0=vpack, scalar1=1e-6, scalar2=None,
                                            op0=ALU.add)
                    xh = wp.tile([128, 2], F32, tag="ntxh")
                    nc.vector.tensor_scalar(out=xh, in0=a, scalar1=0.5, scalar2=None,
                                            op0=ALU.mult)
                    ti = wp.tile([128, 2], I32, tag="nti")
                    nc.vector.tensor_scalar(out=ti, in0=a.bitcast(I32), scalar1=1,
                                            scalar2=None, op0=ALU.logical_shift_right)
                    nc.vector.tensor_scalar(out=ti, in0=ti, scalar1=-1, scalar2=None,
                                            op0=ALU.bitwise_xor)
                    nc.vector.tensor_scalar(out=ti, in0=ti, scalar1=0x5f3759df + 1,
                                            scalar2=None, op0=ALU.add)
                    y = rstd
                    nc.vector.tensor_copy(out=y, in_=ti.bitcast(F32))
                    t1 = wp.tile([128, 2], F32, tag="ntt1")
                    for _ in range(3):
                        nc.vector.tensor_tensor(out=t1, in0=y, in1=y, op=ALU.mult)
                        nc.vector.tensor_tensor(out=t1, in0=t1, in1=xh, op=ALU.mult)
                        nc.vector.tensor_scalar(out=t1, in0=t1, scalar1=-1.0, scalar2=1.5,
                                                op0=ALU.mult, op1=ALU.add)
                        nc.vector.tensor_tensor(out=y, in0=y, in1=t1, op=ALU.mult)
                hln = wp.tile([128, 2, D], F32, tag="hln")
                for t, (toff, tsz) in enumerate(TCH):
                    nc.vector.tensor_scalar(out=hln[0:tsz, t, :], in0=h_sb[0:tsz, t, :],
                                            scalar1=mv[t][0:tsz, 0:1],
                                            scalar2=rstd[0:tsz, t:t + 1],
                                            op0=ALU.subtract, op1=ALU.mult)
                pt = psp.tile([128, 2, 256], F32, tag="ps")
                for kc in range(2):
                    for t, (toff, tsz) in enumerate(TCH):
                        nc.tensor.transpose(pt[0:128, kc, toff:toff + tsz],
                                            hln[0:tsz, t, kc * 128:(kc + 1) * 128],
                                            ident[0:tsz, 0:tsz])
                hlnT = wp.tile([128, 2, 256], F32R, tag="hlnT")
                for kc in range(2):
                    nc.vector.tensor_copy(out=hlnT[:, kc, 0:N], in_=pt[:, kc, 0:N])
                return hlnT

            # ================= in_proj =================
            with tc.tile_pool(name="inp", bufs=1) as inp, \
                 tc.tile_pool(name="psA", bufs=2, space="PSUM") as psA:
                xt_sb = inp.tile([128, NEC, N], F32R)
                win_sb = inp.tile([128, NEC, D], F32R)
                pos_sb = inp.tile([128, 2, D], F32)
                for c, (off, sz) in enumerate(ECH):
                    nc.sync.dma_start(out=xt_sb[0:sz, c, :], in_=xt_d[off:off + sz, :])
                    nc.sync.dma_start(out=win_sb[0:sz, c, :], in_=win_d[off:off + sz, :])
                for t, (toff, tsz) in enumerate(TCH):
                    nc.sync.dma_start(out=pos_sb[0:tsz, t, :], in_=pos_d[toff:toff + tsz, :])
                for t, (toff, tsz) in enumerate(TCH):
                    ph = psA.tile([128, D], F32, tag="ph")
                    for c, (off, sz) in enumerate(ECH):
                        if tsz == 128:
                            nc.tensor.matmul(ph[0:tsz, :], xt_sb[0:sz, c, toff:toff + tsz],
                                             win_sb[0:sz, c, :],
                                             start=(c == 0), stop=(c == NEC - 1))
                        else:
                            nc.tensor.matmul(ph[0:tsz, :],
                                             xt_sb.bitcast(F32)[0:sz, c, toff:toff + tsz],
                                             win_sb.bitcast(F32)[0:sz, c, :],
                                             start=(c == 0), stop=(c == NEC - 1))
                    nc.vector.tensor_tensor(out=h_sb[0:tsz, t, :], in0=ph[0:tsz, :],
                                            in1=pos_sb[0:tsz, t, :], op=ALU.add)
                dump_h("dbg_h0")

            # ================= backbone =================
            with tc.tile_pool(name="wpool", bufs=2) as wpool, \
                 tc.tile_pool(name="tpool", bufs=2) as tpool, \
                 tc.tile_pool(name="psB", bufs=6, space="PSUM") as psB, \
                 tc.tile_pool(name="psS", bufs=2, space="PSUM") as psSm:
                for l in range(LAYERS):
                    wd = LW[l]
                    wq = wpool.tile([128, 2, D], F32R, tag="wq")
                    wk = wpool.tile([128, 2, D], F32R, tag="wk")
                    wv = wpool.tile([128, 2, D], F32R, tag="wv")
                    wo = wpool.tile([128, 2, D], F32R, tag="wo")
                    w1 = wpool.tile([128, 2, BB_H], F32R, tag="w1")
                    w2 = wpool.tile([128, 8, D], F32R, tag="w2")
                    for w_sb, w_dr in [(wq, wd["wq"]), (wk, wd["wk"]), (wv, wd["wv"]),
                                       (wo, wd["wo"]), (w1, wd["w1"]), (w2, wd["w2"])]:
                        nc.sync.dma_start(out=w_sb, in_=w_dr.rearrange("(c p) m -> p c m", p=128))
                    cq = wpool.tile([128, 2], F32, tag="cq")
                    nc.sync.dma_start(out=cq, in_=col2(wd["cq"]))
                    ck = wpool.tile([128, 2], F32, tag="ck")
                    nc.sync.dma_start(out=ck, in_=col2(wd["ck"]))
                    c1 = wpool.tile([128, 8], F32, tag="c1")
                    nc.sync.dma_start(out=c1, in_=col2(wd["c1"]))
                    cv = wpool.tile([1, D], F32, tag="cv")
                    nc.sync.dma_start(out=cv, in_=row(wd["cv"]))
                    co = wpool.tile([1, D], F32, tag="co")
                    nc.sync.dma_start(out=co, in_=row(wd["co"]))
                    c2 = wpool.tile([1, D], F32, tag="c2")
                    nc.sync.dma_start(out=c2, in_=row(wd["c2"]))

                    # --- attention ---
                    hlnT = layer_norm(tpool, psB)
                    qT = tpool.tile([128, 2, N], F32, tag="qT")
                    kT = tpool.tile([128, 2, N], F32, tag="kT")
                    for w_sb, c_sb, out_sb in [(wq, cq, qT), (wk, ck, kT)]:
                        pq = psB.tile([128, 2, 256], F32, tag="ps")
                        for mc in range(2):
                            for kc in range(2):
                                nc.tensor.matmul(pq[:, mc, :],
                                                 w_sb[:, kc, mc * 128:(mc + 1) * 128],
                                                 hlnT[:, kc, :], start=(kc == 0), stop=(kc == 1))
                        for mc in range(2):
                            nc.vector.tensor_scalar(out=out_sb[:, mc, :], in0=pq[:, mc, 0:N],
                                                    scalar1=c_sb[:, mc:mc + 1], scalar2=None,
                                                    op0=ALU.add)
                    pv = psB.tile([128, 2, 256], F32, tag="ps")
                    for t, (toff, tsz) in enumerate(TCH):
                        for kc in range(2):
                            if tsz == 128:
                                nc.tensor.matmul(pv[0:tsz, t, :],
                                                 hlnT[:, kc, toff:toff + tsz],
                                                 wv[:, kc, :], start=(kc == 0), stop=False)
                            else:
                                nc.tensor.matmul(pv[0:tsz, t, :],
                                                 hlnT.bitcast(F32)[:, kc, toff:toff + tsz],
                                                 wv.bitcast(F32)[:, kc, :],
                                                 start=(kc == 0), stop=False)
                        nc.tensor.matmul(pv[0:tsz, t, :], ones_row[0:1, 0:tsz], cv,
                                         start=False, stop=True)
                    v_sb = tpool.tile([128, 2, D], F32, tag="v_sb")
                    nc.vector.tensor_copy(out=v_sb, in_=pv)

                    ET = tpool.tile([128, 2, HEADS, N], F32, tag="ET")
                    for hh in range(HEADS):
                        hp, mc = 64 * (hh % 2), hh // 2
                        pss = psB.tile([128, 2, 256], F32, tag="ps")
                        for t2, (t2off, t2sz) in enumerate(TCH):
                            nc.tensor.matmul(pss[0:t2sz, t2, 0:N],
                                             kT[hp:hp + 64, mc, t2off:t2off + t2sz],
                                             qT[hp:hp + 64, mc, :], start=True, stop=True)
                        nc.scalar.activation(out=ET[:, :, hh, :], in_=pss[:, :, 0:N],
                                             func=AF.Exp, bias=0.0, scale=1.0)
                    for hpair in range(2):
                        prs = psSm.tile([1, 2, 256], F32, tag="pr")
                        for hi in range(2):
                            hh = hpair * 2 + hi
                            for t2, (t2off, t2sz) in enumerate(TCH):
                                nc.tensor.matmul(prs[0:1, hi, 0:N], ones_col[0:t2sz, 0:1],
                                                 ET[0:t2sz, t2, hh, :],
                                                 start=(t2 == 0), stop=(t2 == 1))
                        rsr = tpool.tile([1, 2, N], F32, tag="rsr")
                        nc.vector.reciprocal(out=rsr, in_=prs[0:1, :, 0:N])
                        pbc = psB.tile([128, 2, 256], F32, tag="ps")
                        for hi in range(2):
                            nc.tensor.matmul(pbc[:, hi, 0:N], ones_row[0:1, 0:128],
                                             rsr[0:1, hi, :], start=True, stop=True)
                        for t2 in range(2):
                            nc.vector.tensor_tensor(
                                out=ET[:, t2, hpair * 2:hpair * 2 + 2, :],
                                in0=ET[:, t2, hpair * 2:hpair * 2 + 2, :],
                                in1=pbc[:, :, 0:N], op=ALU.mult)
                    po = psB.tile([128, 2, 256], F32, tag="ps")
                    for hh in range(HEADS):
                        hp, s = 64 * (hh % 2), hh // 2
                        for t2, (t2off, t2sz) in enumerate(TCH):
                            nc.tensor.matmul(po[hp:hp + 64, s, 0:N],
                                             v_sb[0:t2sz, t2, hh * 64:hh * 64 + 64],
                                             ET[0:t2sz, t2, hh, :],
                                             start=(t2 == 0), stop=(t2 == 1),
                                             tile_position=(0, hp))
                    oT = tpool.tile([128, 2, N], F32R, tag="oT")
                    nc.vector.tensor_copy(out=oT, in_=po[:, :, 0:N])
                    pa = psB.tile([128, 2, 256], F32, tag="ps")
                    for t, (toff, tsz) in enumerate(TCH):
                        for kc in range(2):
                            if tsz == 128:
                                nc.tensor.matmul(pa[0:tsz, t, :], oT[:, kc, toff:toff + tsz],
                                                 wo[:, kc, :], start=(kc == 0), stop=False)
                            else:
                                nc.tensor.matmul(pa[0:tsz, t, :],
                                                 oT.bitcast(F32)[:, kc, toff:toff + tsz],
                                                 wo.bitcast(F32)[:, kc, :],
                                                 start=(kc == 0), stop=False)
                        nc.tensor.matmul(pa[0:tsz, t, :], ones_row[0:1, 0:tsz], co,
                                         start=False, stop=True)
                    for t, (toff, tsz) in enumerate(TCH):
                        nc.vector.tensor_tensor(out=h_sb[0:tsz, t, :], in0=h_sb[0:tsz, t, :],
                                                in1=pa[0:tsz, t, :], op=ALU.add)

                    # --- mlp ---
                    hln2T = layer_norm(tpool, psB)
                    gm1T = tpool.tile([128, 8, N], F32R, tag="gm1T")
                    for mc in range(8):
                        pm = psB.tile([128, 256], F32, tag="ps")
                        for kc in range(2):
                            nc.tensor.matmul(pm[:, :],
                                             w1[:, kc, mc * 128:(mc + 1) * 128],
                                             hln2T[:, kc, :],
                                             start=(kc == 0), stop=(kc == 1))
                        nc.scalar.activation(out=gm1T[:, mc, :], in_=pm[:, 0:N],
                                             func=AF.Gelu_apprx_tanh,
                                             bias=c1[:, mc:mc + 1], scale=1.0)
                    pa2 = psB.tile([128, 2, 256], F32, tag="ps")
                    for t, (toff, tsz) in enumerate(TCH):
                        for kc in range(8):
                            if tsz == 128:
                                nc.tensor.matmul(pa2[0:tsz, t, :], gm1T[:, kc, toff:toff + tsz],
                                                 w2[:, kc, :], start=(kc == 0), stop=False)
                            else:
                                nc.tensor.matmul(pa2[0:tsz, t, :],
                                                 gm1T.bitcast(F32)[:, kc, toff:toff + tsz],
                                                 w2.bitcast(F32)[:, kc, :],
                                                 start=(kc == 0), stop=False)
                        nc.tensor.matmul(pa2[0:tsz, t, :], ones_row[0:1, 0:tsz], c2,
                                         start=False, stop=True)
                    for t, (toff, tsz) in enumerate(TCH):
                        nc.vector.tensor_tensor(out=h_sb[0:tsz, t, :], in0=h_sb[0:tsz, t, :],
                                                in1=pa2[0:tsz, t, :], op=ALU.add)
                    dump_h(f"dbg_h{l + 1}")

                # ============ final LN + projections (inside backbone pools) ============
                wf_sb = sb.tile([128, 2, D], F32R)
                nc.sync.dma_start(out=wf_sb, in_=wf_d.rearrange("(c p) m -> p c m", p=128))
                cf_sb = sb.tile([128, 2], F32)
                nc.sync.dma_start(out=cf_sb, in_=col2(cf_d))
                w1a_sb = sb.tile([128, 2, MLP_H], F32R)
                nc.sync.dma_start(out=w1a_sb, in_=w1a_d.rearrange("(c p) m -> p c m", p=128))
                w1b_sb = sb.tile([128, 2, MLP_H], F32R)
                nc.sync.dma_start(out=w1b_sb, in_=w1b_d.rearrange("(c p) m -> p c m", p=128))
                cu_sb = sb.tile([128, 2], F32)
                nc.sync.dma_start(out=cu_sb, in_=col2(cu_d))

                hlnfT = layer_norm(tpool, psB, use_newton=True)
                uT = sb.tile([128, 2, N], F32)
                vT = sb.tile([128, 2, N], F32)
                hfT = sb.tile([128, 2, 256], F32R)
                pf = psB.tile([128, 2, 256], F32, tag="ps")
                for mc in range(2):
                    for kc in range(2):
                        nc.tensor.matmul(pf[:, mc, :], wf_sb[:, kc, mc * 128:(mc + 1) * 128],
                                         hlnfT[:, kc, :], start=(kc == 0), stop=(kc == 1))
                for mc in range(2):
                    nc.vector.tensor_scalar(out=hfT[:, mc, 0:N], in0=pf[:, mc, 0:N],
                                            scalar1=cf_sb[:, mc:mc + 1], scalar2=None,
                                            op0=ALU.add)
                pu = psB.tile([128, 2, 256], F32, tag="ps")
                pvv = psB.tile([128, 2, 256], F32, tag="ps")
                for mc in range(2):
                    for kc in range(2):
                        nc.tensor.matmul(pu[:, mc, :], w1a_sb[:, kc, mc * 128:(mc + 1) * 128],
                                         hfT[:, kc, :], start=(kc == 0), stop=(kc == 1))
                        nc.tensor.matmul(pvv[:, mc, :], w1b_sb[:, kc, mc * 128:(mc + 1) * 128],
                                         hfT[:, kc, :], start=(kc == 0), stop=(kc == 1))
                for mc in range(2):
                    nc.vector.tensor_scalar(out=uT[:, mc, :], in0=pu[:, mc, 0:N],
                                            scalar1=cu_sb[:, mc:mc + 1], scalar2=None,
                                            op0=ALU.add)
                nc.vector.tensor_copy(out=vT, in_=pvv[:, :, 0:N])
                dump_T("dbg_hfT", hfT)
                dump_T("dbg_uT", uT)
                dump_T("dbg_vT", vT)

            # ================= pairwise MLP =================
            w2p_sb = sb.tile([128, 2, MLP_H], F32R)
            nc.sync.dma_start(out=w2p_sb, in_=w2p_d.rearrange("(c p) m -> p c m", p=128))
            b2p_sb = sb.tile([128, 2], F32)
            nc.sync.dma_start(out=b2p_sb, in_=col2(b2p_d))
            b2pr_sb = sb.tile([1, MLP_H], F32R)
            nc.sync.dma_start(out=b2pr_sb, in_=b2pr_d.rearrange("(o p) -> o p", o=1))
            w3b_sb = sb.tile([128, 8, 4], F32R)
            nc.sync.dma_start(out=w3b_sb, in_=w3blk_d)
            maskh_sb = sb.tile([128, 2, N], F32)
            diagm_sb = sb.tile([128, 2, N], F32)
            for t, (toff, tsz) in enumerate(TCH):
                nc.sync.dma_start(out=maskh_sb[0:tsz, t, :], in_=maskh_d[toff:toff + tsz, :])
                nc.sync.dma_start(out=diagm_sb[0:tsz, t, :], in_=diagm_d[toff:toff + tsz, :])

            L_sb = sb.tile([128, 2, N], F32)
            with tc.tile_pool(name="zp", bufs=2) as zp, \
                 tc.tile_pool(name="gzp", bufs=2) as gzp, \
                 tc.tile_pool(name="g2p", bufs=3) as g2p, \
                 tc.tile_pool(name="sc4", bufs=3) as sc4p, \
                 tc.tile_pool(name="pps", bufs=3, space="PSUM") as pps, \
                 tc.tile_pool(name="plt", bufs=2, space="PSUM") as plt:
                for ib in range(N // IBLK):
                    z = zp.tile([128, 2, IBLK * N], F32)
                    gz = gzp.tile([128, 2, IBLK * N], F32R)
                    for ii in range(IBLK):
                        i = ib * IBLK + ii
                        for kc in range(2):
                            nc.vector.tensor_scalar(out=z[:, kc, ii * N:(ii + 1) * N],
                                                    in0=vT[:, kc, :],
                                                    scalar1=uT[:, kc, i:i + 1],
                                                    scalar2=None, op0=ALU.add)
                    for ih in range(4):
                        sl = slice(ih * (IBLK // 4) * N, (ih + 1) * (IBLK // 4) * N)
                        nc.scalar.activation(out=gz[:, :, sl], in_=z[:, :, sl],
                                             func=AF.Gelu_apprx_tanh, bias=0.0, scale=1.0)
                    for g4 in range(IBLK // 4):
                        g2sb = g2p.tile([128, 2, 4, 256], F32R)
                        for pp in range(2):
                            # [128, mc, 512-pad]: one accumulation group per bank,
                            # N=384 covers two consecutive i's in one matmul
                            ps2 = pps.tile([128, 2, 512], F32)
                            iioff = (g4 * 4 + pp * 2) * N
                            for mc in range(2):
                                for kc in range(2):
                                    nc.tensor.matmul(
                                        ps2[:, mc, 0:2 * N],
                                        w2p_sb[:, kc, mc * 128:(mc + 1) * 128],
                                        gz[:, kc, iioff:iioff + 2 * N],
                                        start=(kc == 0), stop=False)
                                nc.tensor.matmul(
                                    ps2[:, mc, 0:2 * N],
                                    b2pr_sb[0:1, mc * 128:(mc + 1) * 128],
                                    ones_row_r[0:1, 0:2 * N],
                                    start=False, stop=True)
                            nc.scalar.activation(
                                out=g2sb[:, :, pp * 2:pp * 2 + 2, 0:N],
                                in_=ps2[:, :, 0:2 * N].rearrange(
                                    "p a (b n) -> p a b n", n=N),
                                func=AF.Gelu_apprx_tanh, bias=0.0, scale=1.0)
                        lt = plt.tile([4, 256], F32)
                        for kc8 in range(8):
                            ii, mc = kc8 // 2, kc8 % 2
                            nc.tensor.matmul(lt[0:4, :], w3b_sb[:, kc8, :],
                                             g2sb[:, mc, ii, :],
                                             start=(kc8 == 0), stop=(kc8 == 7))
                        i0 = ib * IBLK + g4 * 4
                        t, r = i0 // 128, i0 % 128
                        sc4 = sc4p.tile([4, N], F32)
                        nc.vector.tensor_copy(out=sc4, in_=lt[0:4, 0:N])
                        nc.sync.dma_start(out=L_sb[r:r + 4, t, :], in_=sc4)

            if _debug:
                for t, (toff, tsz) in enumerate(TCH):
                    nc.sync.dma_start(out=DBG["dbg_L"][toff:toff + tsz, :], in_=L_sb[0:tsz, t, :])
            # ================= symmetrize + logits + P0 + Sinkhorn =================
            with tc.tile_pool(name="psC", bufs=2, space="PSUM") as psC, \
                 tc.tile_pool(name="psV", bufs=2, space="PSUM") as psV:
                psL = psC.tile([128, 2, 256], F32, tag="ps")
                for t_out, (joff, jsz) in enumerate(TCH):
                    for t_in, (ioff, isz) in enumerate(TCH):
                        nc.tensor.transpose(psL[0:jsz, t_out, ioff:ioff + isz],
                                            L_sb[0:isz, t_in, joff:joff + jsz],
                                            ident[0:isz, 0:isz])
                Lsym = sb.tile([128, 2, N], F32)
                nc.vector.tensor_tensor(out=Lsym, in0=L_sb, in1=psL[:, :, 0:N], op=ALU.add)
                Lmh = sb.tile([128, 2, N], F32)
                nc.vector.tensor_tensor(out=Lmh, in0=Lsym, in1=maskh_sb, op=ALU.mult)
                lo = sb.tile([128, 2, N], F32)
                nc.vector.tensor_tensor(out=lo, in0=Lmh, in1=diagm_sb, op=ALU.add)
                for t, (toff, tsz) in enumerate(TCH):
                    nc.sync.dma_start(out=logits_d[toff:toff + tsz, :], in_=lo[0:tsz, t, :])

                mrow = sb.tile([128, 2], F32)
                for t, (toff, tsz) in enumerate(TCH):
                    nc.vector.reduce_max(out=mrow[0:tsz, t:t + 1], in_=Lmh[0:tsz, t, :],
                                         axis=AX.X)
                pmr = psV.tile([1, 2, 128], F32, tag="pv")
                for t, (toff, tsz) in enumerate(TCH):
                    nc.tensor.transpose(pmr[0:1, t, 0:tsz], mrow[0:tsz, t:t + 1],
                                        ident[0:tsz, 0:tsz])
                mjoin = sb.tile([1, N], F32)
                nc.vector.tensor_copy(out=mjoin[0:1, 0:128], in_=pmr[0:1, 0, :])
                nc.vector.tensor_copy(out=mjoin[0:1, 128:192], in_=pmr[0:1, 1, 0:64])
                gmax = sb.tile([1, 1], F32)
                nc.vector.reduce_max(out=gmax, in_=mjoin, axis=AX.X)
                pgb = psV.tile([128, 1], F32, tag="pg")
                nc.tensor.matmul(pgb, ones_row[0:1, 0:128], gmax, start=True, stop=True)
                negb = sb.tile([128, 1], F32)
                nc.vector.tensor_scalar(out=negb, in0=pgb, scalar1=-20.0, scalar2=None,
                                        op0=ALU.mult)

                P0 = sb.tile([128, 2, N], F32)
                nc.scalar.activation(out=P0, in_=lo, func=AF.Exp, bias=negb, scale=20.0)

                ccol0 = sb.tile([128, 2], F32)
                nc.vector.memset(ccol0, 1.0)
                cur = ccol0
                rcol = ccol = None
                for it in range(SINK_ITERS):
                    for half in range(2):
                        pd = psV.tile([128, 2], F32, tag="pd")
                        for mc, (moff, msz) in enumerate(TCH):
                            for kc, (koff, ksz) in enumerate(TCH):
                                nc.tensor.matmul(pd[0:msz, mc:mc + 1],
                                                 P0[0:ksz, kc, moff:moff + msz],
                                                 cur[0:ksz, kc:kc + 1],
                                                 start=(kc == 0), stop=(kc == 1))
                        nxt = sb.tile([128, 2], F32, tag=f"rc{half}{it % 2}")
                        nc.vector.reciprocal(out=nxt, in_=pd)
                        if half == 0:
                            rcol = nxt
                        else:
                            ccol = nxt
                        cur = nxt

                prr = psV.tile([1, 2, 128], F32, tag="pv")
                pcc = psV.tile([1, 2, 128], F32, tag="pv")
                for t, (toff, tsz) in enumerate(TCH):
                    nc.tensor.transpose(prr[0:1, t, 0:tsz], rcol[0:tsz, t:t + 1],
                                        ident[0:tsz, 0:tsz])
                    nc.tensor.transpose(pcc[0:1, t, 0:tsz], ccol[0:tsz, t:t + 1],
                                        ident[0:tsz, 0:tsz])
                rrow = sb.tile([1, N], F32)
                crow = sb.tile([1, N], F32)
                for t, (toff, tsz) in enumerate(TCH):
                    nc.vector.tensor_scalar(out=rrow[0:1, toff:toff + tsz],
                                            in0=prr[0:1, t, 0:tsz],
                                            scalar1=0.5, scalar2=None, op0=ALU.mult)
                    nc.vector.tensor_copy(out=crow[0:1, toff:toff + tsz],
                                          in_=pcc[0:1, t, 0:tsz])
                psS = psC.tile([128, 2, 256], F32, tag="ps")
                for mc, (moff, msz) in enumerate(TCH):
                    nc.tensor.matmul(psS[0:msz, mc, 0:N], rrow[0:1, moff:moff + msz], crow,
                                     start=True, stop=False)
                    nc.tensor.matmul(psS[0:msz, mc, 0:N], crow[0:1, moff:moff + msz], rrow,
                                     start=False, stop=True)
                x0sb = sb.tile([128, 2, N], F32)
                nc.vector.tensor_tensor(out=x0sb, in0=P0, in1=psS[:, :, 0:N], op=ALU.mult)
                for t, (toff, tsz) in enumerate(TCH):
                    nc.sync.dma_start(out=x0_d[toff:toff + tsz, :], in_=x0sb[0:tsz, t, :])

    nc.compile()
    return nc


_PROGRAM = None


def kernel(x, timesteps, params, _trace=False):
    global _PROGRAM
    in_maps = _host_prep(x, timesteps, params)
    if _PROGRAM is None:
        _PROGRAM = build_program()
    res = run_bass_kernel_spmd(_PROGRAM, in_maps, list(range(B)), trace=_trace)
    _PROFILE["last"] = res
    logits = np.stack([res.results[b]["logits"] for b in range(B)])
    x0 = np.stack([res.results[b]["x0"] for b in range(B)])
    return logits, x0
